# revision 1
# baseline (speedup 1.0000x reference)
"""Multi-head attention Trainium2 kernel (8-core SPMD).

Problem: B=2, S=2048, EMBED=1024, HEADS=16, HEAD_DIM=64.
  v,k,q = split_heads(X) @ W{v,k,q}.T  (per-head, shared 64x64 weights)
  out   = softmax(q k^T / 8) v ; merge heads ; out @ Wo.T + bo

Sharding: core c -> batch b=c//4, query rows [qi*512, qi*512+512), qi=c%4.
Each core computes all 16 heads for its 512 query rows; K/V projections are
replicated inside each batch group so NO collectives are needed, and the
output is a disjoint row-slice gather on the host.

Design (v1 was PE-bound at ~222us; this version is ACT-bound, the
fundamental floor for this problem):
  - ACT (exp) floor: 16 heads x 2048k x 512q = 16.7M exp elements/core
    at 1 elem/cycle/lane @1.2GHz = ~143us busy (FD=1024 instructions,
    (N+318)/1.2ns each). Everything else hides under it.
  - Heads of a pair processed SEQUENTIALLY; the K=64-contraction score
    matmuls are issued as row-tiled pairs (kb-even in PE row groups 0-1,
    kb-odd concurrently in groups 2-3; operands live at the matching
    partition halves via cheap DVE 4x-mode dup copies). ~2x S throughput.
  - One flat chunk stream, software-pipelined one deep (S+exp at n, PV
    at n-1) so the next chunk's S pair sits in the PE FIFO ahead of the
    previous chunk's PV; ACT stays fed across chunk/head/pair boundaries.
    (Two deep measured WORSE: exp slowed 1113->1335ns from extra
    concurrent PSUM traffic.)
  - Next-pair DMA loads + K/V/Q projections and the fc_out partial
    rounds are emitted in small pieces at fixed chunk slots (the Tile
    scheduler keeps per-engine FIFO order ~= emission order; lumping
    them stalls the PE FIFO and starves ACT).
  - V_aug [128k, 65] per (kb, head) carries a ones column so PSUM row 64
    accumulates the softmax denominator during PV. Normalization:
    denominator row -> partition 0 (standard DVE copy honors partition
    offsets), custom-DVE reciprocal, gpsimd partition_broadcast,
    multiply straight from PSUM rows 0-63 -> merged fp16.
  - fc_out: partial rounds every 2 pairs (2 MMs into a mix PSUM tile +
    DVE add into fp16 SBUF accumulators, 4 tiles on each of 2 pairs);
    the final round folds acc back in via an identity-stationary matmul
    and evacuates alternately on ACT (idle once exps end) and DVE, with
    the norm-independent matmuls emitted first to keep the PE warm.
  - Startup: exp ACT-table preloaded via a dummy activation; ~4us of
    junk fp32 matmuls during the DMA wait flip the HAM clock gate to
    8/8 before the first real matmul; pair 0 skips the A-side dup tiles
    (serial S in one row group) to shorten the critical DVE chain.
  - PSUM budget: scores 2x[128,1024]f32 (4 banks) + po 2 + mix 2 = 8.
"""

import os
import sys

sys.path.insert(0, "/opt/trn_rl_repo")

import numpy as np

import concourse.bass as bass
import concourse.mybir as mybir
import concourse.tile as tile
from concourse import bacc
from concourse.bass_utils import run_bass_kernel_spmd

B = 2
S = 2048
E = 1024
H = 16
D = 64
SQ = 512          # query rows per core
NCORES = 8
NPAIR = 8         # head pairs
KBLK = 16         # 128-row key blocks
FP = mybir.dt.float32

KDT = os.environ.get("KERNEL_DT", "fp16")  # fp16 | bf16 | f32r | fp32


def build_nc(kdt=None):
    kdt = kdt or KDT
    MD = {"fp16": mybir.dt.float16, "bf16": mybir.dt.bfloat16,
          "f32r": mybir.dt.float32r, "fp32": FP}[kdt]  # matmul operand dtype
    nc = bacc.Bacc("TRN2", target_bir_lowering=False, debug=False)

    ident = nc.dram_tensor("ident", [128, 128], MD, kind="ExternalInput").ap()
    xq_t = nc.dram_tensor("xq_t", [E, SQ], MD, kind="ExternalInput").ap()
    xk_t = nc.dram_tensor("xk_t", [E, S], MD, kind="ExternalInput").ap()
    xv_t = nc.dram_tensor("xv_t", [E, S], MD, kind="ExternalInput").ap()
    wq_bd = nc.dram_tensor("wq_bd", [128, 128], MD, kind="ExternalInput").ap()
    wk_bd = nc.dram_tensor("wk_bd", [128, 128], MD, kind="ExternalInput").ap()
    wv_bd = nc.dram_tensor("wv_bd", [128, 128], MD, kind="ExternalInput").ap()
    wo_t = nc.dram_tensor("wo_t", [E, E], MD, kind="ExternalInput").ap()
    bo = nc.dram_tensor("bo", [1, E], FP, kind="ExternalInput").ap()
    out = nc.dram_tensor("out", [SQ, E], FP, kind="ExternalOutput").ap()

    with tile.TileContext(nc) as tc:
        _body(tc, xq_t, xk_t, xv_t, wq_bd, wk_bd, wv_bd, wo_t, bo, ident,
              out, MD)
    nc.compile()
    return nc


def _body(tc, xq_t, xk_t, xv_t, wq_bd, wk_bd, wv_bd, wo_t, bo, ident,
          out, MD):
    """Software-pipelined emission. The Tile scheduler keeps per-engine
    FIFO order ~= emission order, so next-pair DMA loads and projections
    and the fc_out partial rounds are emitted INTERLEAVED into the
    attention chunk stream of the current pair; otherwise they serialize
    at pair boundaries behind the norm chain (observed: 21us ACT stalls
    + PE idle >3.4us -> HAM re-throttle)."""
    from contextlib import ExitStack
    nc = tc.nc
    Exp = mybir.ActivationFunctionType.Exp

    ctx = ExitStack()
    with ctx:
        wp = ctx.enter_context(tc.tile_pool(name="w", bufs=1))
        xkp = ctx.enter_context(tc.tile_pool(name="xk", bufs=3))
        xvp = ctx.enter_context(tc.tile_pool(name="xv", bufs=3))
        xqp = ctx.enter_context(tc.tile_pool(name="xq", bufs=3))
        ktp = ctx.enter_context(tc.tile_pool(name="kt", bufs=2))
        kdp = ctx.enter_context(tc.tile_pool(name="kd", bufs=2))  # dup halves
        vp = ctx.enter_context(tc.tile_pool(name="v", bufs=2))
        qtp = ctx.enter_context(tc.tile_pool(name="qt", bufs=2))
        qdp = ctx.enter_context(tc.tile_pool(name="qd", bufs=2))
        ptp = ctx.enter_context(tc.tile_pool(name="pt", bufs=6))
        mgp = ctx.enter_context(tc.tile_pool(name="mg", bufs=4))
        dnp = ctx.enter_context(tc.tile_pool(name="dn", bufs=4))
        acp = ctx.enter_context(tc.tile_pool(name="ac", bufs=1))
        obp = ctx.enter_context(tc.tile_pool(name="ob", bufs=4))
        ps_s = ctx.enter_context(tc.tile_pool(name="ps_s", bufs=2, space="PSUM"))
        ps_o = ctx.enter_context(tc.tile_pool(name="ps_o", bufs=2, space="PSUM"))
        ps_m = ctx.enter_context(tc.tile_pool(name="ps_m", bufs=2, space="PSUM"))

        # ---- weights / bias; order tuned so the startup critical path
        # (wq -> xq -> qproj, wk -> xk -> kproj -> first S -> first exp)
        # is front-loaded on the single DMA queue ----
        wq = wp.tile([128, 128], MD, tag="wq")
        wk = wp.tile([128, 128], MD, tag="wk")
        wv = wp.tile([128, 128], MD, tag="wv")
        nc.sync.dma_start(wq[:], wq_bd)
        nc.sync.dma_start(wk[:], wk_bd)
        nbias = wp.tile([128, 1], FP, tag="nbias")
        nc.gpsimd.memset(nbias[:], -4.0)
        ones16 = wp.tile([128, KBLK], FP, tag="ones16")
        nc.gpsimd.memset(ones16[:], 1.0)
        # dummy activation: preload the exp table set (~2.7us) during the
        # startup DMAs instead of on the first real exp
        warm = wp.tile([1, 8], FP, tag="warm")
        nc.gpsimd.memset(warm[:], 0.0)
        nc.scalar.activation(warm[:], warm[:], Exp, scale=1.0,
                             bias=nbias[0:1, 0:1])
        # dummy fp32 matmuls on junk data: ~4us of sustained PE activity
        # during the startup DMA wait flips the HAM clock gate to 8/8, so
        # the first REAL matmuls run at 2.4GHz instead of 1.2
        wmt = wp.tile([128, 512], FP, tag="wmt")
        nc.gpsimd.memset(wmt[:], 0.25)
        for w_ in range(2):
            # 2 cold fp32 matmuls = ~3.4us, exactly one HAM SHORT window;
            # more would delay the first real matmul in the PE FIFO
            psw = ps_m.tile([128, 512], FP, tag="mix", name="psw")
            nc.tensor.matmul(psw[:], lhsT=wmt[:, 0:128], rhs=wmt[:],
                             start=True, stop=True)

        wo_tiles = [wp.tile([128, E], MD, tag=f"wo{et}", name=f"wo{et}")
                    for et in range(8)]
        id_t = wp.tile([128, 128], MD, tag="id_t")
        # fc_out SBUF accumulators (fp16 so the final round can fold them
        # into PSUM via an identity-stationary matmul), one per output tile
        acc = [acp.tile([128, 512], MD, tag=f"acc{i}", name=f"acc{i}")
               for i in range(8)]
        merged = {}
        st = {}  # pipelined per-pair tiles

        def emit_loads(p):
            xq = xqp.tile([128, SQ], MD, tag="xq")
            nc.sync.dma_start(xq[:], xq_t[p * 128:(p + 1) * 128, :])
            xk = xkp.tile([128, S], MD, tag="xk")
            for ch in range(4):
                nc.sync.dma_start(xk[:, ch * 512:(ch + 1) * 512],
                                  xk_t[p * 128:(p + 1) * 128,
                                       ch * 512:(ch + 1) * 512])
            xv = xvp.tile([128, S], MD, tag="xv")
            for ch in range(2):
                nc.sync.dma_start(xv[:, ch * 1024:(ch + 1) * 1024],
                                  xv_t[p * 128:(p + 1) * 128,
                                       ch * 1024:(ch + 1) * 1024])
            st["x", p] = (xk, xv, xq)

        def emit_kproj(p, half, dups=True):
            # K^T projection [128(d2), 2048(k)] + chunked dup halves so the
            # active head's K^T exists at BOTH partition halves for the
            # row-tiled S pairs. Pair 0 skips the dups (its S matmuls run
            # serially in one row group) to keep the startup DVE chain short.
            xk = st["x", p][0]
            if half == 0:
                kt = ktp.tile([128, S], MD, tag="kt")
                if dups:
                    kdA = kdp.tile([128, S], MD, tag="kdA")  # even @ 64:128
                    kdB = kdp.tile([128, S], MD, tag="kdB")  # odd @ 0:64
                else:
                    kdA = kdB = None
                st["k", p] = (kt, kdA, kdB)
            else:
                kt, kdA, kdB = st["k", p]
            for ch in (2 * half, 2 * half + 1):
                sl = slice(ch * 512, (ch + 1) * 512)
                ps = ps_m.tile([128, 512], FP, tag="mix")
                nc.tensor.matmul(ps[:], lhsT=wk[:], rhs=xk[:, sl],
                                 start=True, stop=True)
                nc.vector.tensor_copy(kt[:, sl], ps[:])
                if dups:
                    nc.vector.tensor_copy(kdA[64:128, sl], kt[0:64, sl])
                    nc.vector.tensor_copy(kdB[0:64, sl], kt[64:128, sl])

        def emit_vproj(p, half):
            # V natural projection with ones columns (col 64 of each head
            # block, so the softmax denominator lands in po row 64)
            xv = st["x", p][1]
            if half == 0:
                v = vp.tile([128, KBLK * 130], MD, tag="v")
                vr = v[:].rearrange("p (b c) -> p b c", c=130)
                nc.vector.tensor_copy(vr[:, :, 64:65], ones16[:])
                nc.vector.tensor_copy(vr[:, :, 129:130], ones16[:])
                st["v", p] = v
            else:
                v = st["v", p]
            for vg in (2 * half, 2 * half + 1):
                ps = ps_m.tile([128, 512], FP, tag="mix")
                for j in range(4):
                    kb = vg * 4 + j
                    nc.tensor.matmul(ps[:, j * 128:(j + 1) * 128],
                                     lhsT=xv[:, kb * 128:(kb + 1) * 128],
                                     rhs=wv[:], start=True, stop=True)
                src4 = ps[:].rearrange("p (b g c) -> p b g c", g=2, c=64)
                dst4 = v[:, vg * 520:(vg + 1) * 520].rearrange(
                    "p (b g c) -> p b g c", g=2, c=65)[:, :, :, 0:64]
                nc.vector.tensor_copy(dst4, src4)

        def emit_qproj(p, dups=True):
            xq = st["x", p][2]
            qt = qtp.tile([128, SQ], MD, tag="qt")
            psq = ps_m.tile([128, 512], FP, tag="mix")
            nc.tensor.matmul(psq[:], lhsT=wq[:], rhs=xq[:],
                             start=True, stop=True)
            nc.vector.tensor_copy(qt[:], psq[:])
            if dups:
                qdA = qdp.tile([128, SQ], MD, tag="qdA")
                nc.vector.tensor_copy(qdA[64:128, :], qt[0:64, :])
                qdB = qdp.tile([128, SQ], MD, tag="qdB")
                nc.vector.tensor_copy(qdB[0:64, :], qt[64:128, :])
            else:
                qdA = qdB = None
            st["q", p] = (qt, qdA, qdB)

        def emit_fc_tile(pa, pb, i):
            # one fc_out output tile: acc[i] (+)= merged[pa] @ wo[pa]
            #                                  + merged[pb] @ wo[pb]
            sb, nch = i // 2, i % 2
            psf_t = ps_m.tile([128, 512], FP, tag="mix", name="psf")
            psf = psf_t[:]
            nc.tensor.matmul(
                psf,
                lhsT=merged[pa][:, sb * 128:(sb + 1) * 128],
                rhs=wo_tiles[pa][:, nch * 512:(nch + 1) * 512],
                start=True, stop=False, skip_group_check=True)
            nc.tensor.matmul(
                psf,
                lhsT=merged[pb][:, sb * 128:(sb + 1) * 128],
                rhs=wo_tiles[pb][:, nch * 512:(nch + 1) * 512],
                start=False, stop=True, skip_group_check=True)
            if pa == 0:
                nc.vector.tensor_add(acc[i][:], psf,
                                     bo_b[:, nch * 512:(nch + 1) * 512])
            else:
                nc.vector.tensor_add(acc[i][:], acc[i][:], psf)

        # final fc round, split so the norm(7,hp1)-independent matmuls
        # (merged[6], merged[7] top half, identity*acc) keep the PE warm
        # while the last norm chain runs on DVE/GpSimd; evacuation
        # alternates ACT (idle after the last exp) and DVE.
        psfs = {}

        def fc_final_early(i):
            sb, nch = i // 2, i % 2
            if i % 2 == 0:
                big = ps_s.tile([128, 1024], FP, tag="s", name=f"fcf{i}")
                psf = big[:, 0:512]
            else:
                psf_t = ps_m.tile([128, 512], FP, tag="mix", name="psf")
                psf = psf_t[:]
            nc.tensor.matmul(
                psf, lhsT=merged[6][:, sb * 128:(sb + 1) * 128],
                rhs=wo_tiles[6][:, nch * 512:(nch + 1) * 512],
                start=True, stop=False, skip_group_check=True)
            nc.tensor.matmul(
                psf, lhsT=merged[7][0:64, sb * 128:(sb + 1) * 128],
                rhs=wo_tiles[7][0:64, nch * 512:(nch + 1) * 512],
                start=False, stop=False, skip_group_check=True)
            nc.tensor.matmul(
                psf, lhsT=id_t[:], rhs=acc[i][:],
                start=False, stop=False, skip_group_check=True)
            psfs[i] = psf

        def fc_final_late(i):
            sb, nch = i // 2, i % 2
            psf = psfs.pop(i)
            nc.tensor.matmul(
                psf, lhsT=merged[7][64:128, sb * 128:(sb + 1) * 128],
                rhs=wo_tiles[7][64:128, nch * 512:(nch + 1) * 512],
                start=False, stop=True, skip_group_check=True)
            ot = obp.tile([128, 512], FP, tag="ob")
            if i % 2 == 0:
                nc.scalar.copy(ot[:], psf)
            else:
                nc.vector.tensor_copy(ot[:], psf)
            nc.sync.dma_start(
                out[sb * 128:(sb + 1) * 128,
                    nch * 512:(nch + 1) * 512], ot[:])

        def emit_norm(p, hp, po):
            # normalize: denominator row 64 -> partition 0 via a standard
            # copy (honors AP partition offsets; custom-DVE recip and
            # gpsimd broadcast need input physically at partition 0),
            # then multiply po rows 0-63 straight from PSUM (base 0).
            mg = mgp.tile([128, SQ], MD, name=f"m{p}", tag="mg") \
                if hp == 0 else merged[p]
            merged[p] = mg
            dn = dnp.tile([1, 512], FP, tag="dn")
            nc.vector.tensor_copy(dn[0:1, :], po[64:65, :])
            dr = dnp.tile([1, 512], FP, tag="dr")
            nc.vector.reciprocal_approx_fast(dr[0:1, :], dn[0:1, :])
            db = dnp.tile([64, 512], FP, tag="db")
            nc.gpsimd.partition_broadcast(db[:], dr[0:1, :], channels=64)
            nc.vector.tensor_mul(mg[hp * 64:(hp + 1) * 64, :],
                                 po[0:64, :], db[:])

        # ---- prologue: pair 0 fully, pair 1 loads ----
        emit_loads(0)
        nc.sync.dma_start(wv[:], wv_bd)
        bo_row = wp.tile([1, E], FP, tag="bo_row")
        nc.sync.dma_start(bo_row[:], bo)
        bo_b = wp.tile([128, E], FP, tag="bo_b")
        nc.gpsimd.partition_broadcast(bo_b[:], bo_row[0:1, :], channels=128)
        emit_qproj(0, dups=False)
        emit_kproj(0, 0, dups=False)
        emit_kproj(0, 1, dups=False)
        # pair-0 B-side dups only: hp0 runs serial S (PE is HAM-cold anyway)
        # but hp1 (past the cold window) still row-tiles
        kt0_ = st["k", 0][0]
        qt0_ = st["q", 0][0]
        kdB0 = kdp.tile([128, S], MD, tag="kdB")
        nc.vector.tensor_copy(kdB0[0:64, :], kt0_[64:128, :])
        qdB0 = qdp.tile([128, SQ], MD, tag="qdB")
        nc.vector.tensor_copy(qdB0[0:64, :], qt0_[64:128, :])
        st["k", 0] = (kt0_, None, kdB0)
        st["q", 0] = (qt0_, None, qdB0)
        emit_vproj(0, 0)
        emit_vproj(0, 1)
        emit_loads(1)

        # ---- attention: one flat chunk stream, software-pipelined ONE
        # deep (S+exp at n, PV at n-1) - the next chunk's S pair enters the
        # PE FIFO ahead of the previous chunk's PV, keeping ACT fed across
        # chunk, head, and pair boundaries. (A two-deep variant measured
        # WORSE: exp durations grew 1113->1335ns from the extra concurrent
        # PSUM traffic.) Heads sequential; S matmuls issued as row-tiled
        # pairs (kb even in PE rows 0-63, kb odd in 64-127, concurrent on
        # HW); exp FD=1024; PV accumulates po.
        chunks = [(p, hp, c) for p in range(NPAIR) for hp in range(2)
                  for c in range(8)]
        NCH = len(chunks)
        pts, pos = {}, {}

        def stage_S_exp(n):
            p, hp, c = chunks[n]
            if p == 1 and hp == 0 and c == 0:
                # fc weights aren't needed until pair 2's fc round
                for et in range(8):
                    nc.sync.dma_start(wo_tiles[et][:],
                                      wo_t[et * 128:(et + 1) * 128, :])
                nc.sync.dma_start(id_t[:], ident)
            kt, kdA, kdB = st["k", p]
            qt, qdA, qdB = st["q", p]
            if kdA is None and hp == 0:
                # pair 0 head 0: no A-dups; both S matmuls in row group 0
                # (serial on PE - fine, startup is HAM-cold anyway)
                r0 = r1 = slice(0, 64)
                kt0 = kt1 = kt
                qt0 = qt1 = qt
            elif hp == 0:
                kt0, qt0, r0 = kt, qt, slice(0, 64)
                kt1, qt1, r1 = kdA, qdA, slice(64, 128)
            else:
                kt0, qt0, r0 = kdB, qdB, slice(0, 64)
                kt1, qt1, r1 = kt, qt, slice(64, 128)
            kb0, kb1 = 2 * c, 2 * c + 1
            ps = ps_s.tile([128, 1024], FP, tag="s", name=f"s{p}_{hp}_{c}")
            nc.tensor.matmul(
                ps[:, 0:512],
                lhsT=kt0[r0, kb0 * 128:(kb0 + 1) * 128],
                rhs=qt0[r0, :],
                start=True, stop=True)
            nc.tensor.matmul(
                ps[:, 512:1024],
                lhsT=kt1[r1, kb1 * 128:(kb1 + 1) * 128],
                rhs=qt1[r1, :],
                start=True, stop=True)
            # exp(s/8 - 4): the -4 shift cancels in softmax and keeps
            # max P ~= e^7 well inside fp16 range
            pt_ = ptp.tile([128, 1024], MD, name="pt_")
            nc.scalar.activation(pt_[:], ps[:], Exp,
                                 scale=0.125, bias=nbias[:])
            pts[n] = pt_

        def stage_pv(n):
            p, hp, c = chunks[n]
            if c == 0:
                pos[p, hp] = ps_o.tile([65, 512], FP, tag="o",
                                       name=f"po{p}_{hp}")
            po = pos[p, hp]
            v = st["v", p]
            pt_ = pts.pop(n)
            kb0, kb1 = 2 * c, 2 * c + 1
            nc.tensor.matmul(
                po[:],
                lhsT=v[:, kb0 * 130 + hp * 65:kb0 * 130 + hp * 65 + 65],
                rhs=pt_[:, 0:512],
                start=(c == 0), stop=False, skip_group_check=True)
            nc.tensor.matmul(
                po[:],
                lhsT=v[:, kb1 * 130 + hp * 65:kb1 * 130 + hp * 65 + 65],
                rhs=pt_[:, 512:1024],
                start=False, stop=(c == 7), skip_group_check=True)
            if c == 7:
                emit_norm(p, hp, pos.pop((p, hp)))

        def interleave(n):
            # next-pair projections and fc-round tiles, one small piece per
            # chunk slot so no emission point exceeds the per-chunk ACT
            # budget (~1.1us)
            p, hp, c = chunks[n]
            t = hp * 8 + c
            if t == 1 and p < 6:
                emit_loads(p + 2)
                return
            # pair 0 defers its projection slots past the HAM-cold window
            tt = t - 4 if p == 0 else t
            if tt == 2 and p < 7:
                emit_kproj(p + 1, 0)
            elif tt == 3 and p < 7:
                emit_kproj(p + 1, 1)
            elif tt == 4 and p < 7:
                emit_vproj(p + 1, 0)
            elif tt == 5 and p < 7:
                emit_vproj(p + 1, 1)
            elif tt == 6 and p < 7:
                emit_qproj(p + 1)
            elif p == 7 and 2 <= t <= 5:
                # pair 7 has no next-pair projections; run its share of the
                # (4,5) round in the early slots so the final exps stream
                # without interleaved fc work
                emit_fc_tile(4, 5, t + 2)
            elif 7 <= t <= 10 and 2 <= p < 7:
                # fc round for an earlier pair couple, 4 tiles on the even
                # pair and 4 on the odd pair, so no single pair carries the
                # whole 16-matmul round
                if p % 2 == 0:
                    emit_fc_tile(p - 2, p - 1, t - 7)
                else:
                    emit_fc_tile(p - 3, p - 2, t - 3)

        for n in range(NCH + 1):
            if n < NCH:
                stage_S_exp(n)
            if 0 <= n - 1 < NCH:
                stage_pv(n - 1)
            if n < NCH:
                interleave(n)

        # batch the phases: all 4 early groups, then their late+evac (ACT
        # and DVE copies flow in parallel), then the next 4 - the previous
        # one-by-one staggering serialized each early group behind a copy
        for i in range(4):
            fc_final_early(i)
        for i in range(4):
            fc_final_late(i)
        for i in range(4, 8):
            fc_final_early(i)
        for i in range(4, 8):
            fc_final_late(i)


# ---------------------------------------------------------------------------
# host side
# ---------------------------------------------------------------------------

_NC_CACHE = {}


def _get_nc():
    if KDT not in _NC_CACHE:
        _NC_CACHE[KDT] = build_nc(KDT)
    return _NC_CACHE[KDT]


def _np_dt():
    if KDT == "bf16":
        import ml_dtypes
        return ml_dtypes.bfloat16
    if KDT == "fp16":
        return np.float16
    return np.float32


def _bd(w):
    """128x128 block-diag of W.T (two copies)."""
    wt = np.ascontiguousarray(np.asarray(w).T.astype(np.float32))
    o = np.zeros((128, 128), np.float32)
    o[:64, :64] = wt
    o[64:, 64:] = wt
    return o


def kernel(values, keys, queries, Wv, Wk, Wq, Wo, bo):
    values = np.asarray(values, np.float32)
    keys = np.asarray(keys, np.float32)
    queries = np.asarray(queries, np.float32)

    dt = _np_dt()
    ident = np.eye(128, dtype=np.float32).astype(dt)
    wq_bd = _bd(Wq).astype(dt)
    wk_bd = _bd(Wk).astype(dt)
    wv_bd = _bd(Wv).astype(dt)
    wo_t = np.ascontiguousarray(np.asarray(Wo, np.float32).T).astype(dt)
    bo_r = np.ascontiguousarray(np.asarray(bo, np.float32).reshape(1, E))

    xk_t = [np.ascontiguousarray(keys[b].T).astype(dt) for b in range(B)]
    xv_t = [np.ascontiguousarray(values[b].T).astype(dt) for b in range(B)]

    in_maps = []
    for c in range(NCORES):
        b, qi = c // 4, c % 4
        in_maps.append({
            "xq_t": np.ascontiguousarray(
                queries[b, qi * SQ:(qi + 1) * SQ, :].T).astype(dt),
            "xk_t": xk_t[b],
            "xv_t": xv_t[b],
            "wq_bd": wq_bd, "wk_bd": wk_bd, "wv_bd": wv_bd,
            "wo_t": wo_t, "bo": bo_r, "ident": ident,
        })

    nc = _get_nc()
    res = run_bass_kernel_spmd(nc, in_maps, list(range(NCORES)),
                               trace=bool(int(os.environ.get("BASS_TRACE", "0"))))
    full = np.empty((B, S, E), np.float32)
    for c in range(NCORES):
        b, qi = c // 4, c % 4
        full[b, qi * SQ:(qi + 1) * SQ, :] = res.results[c]["out"]
    kernel.last_results = res
    return full



# revision 3
# speedup vs baseline: 1.1140x; 1.1140x over previous
"""Multi-head attention Trainium2 kernel (8-core SPMD), v3: host-folded weights.

Problem: B=2, S=2048, EMBED=1024, HEADS=16, HEAD_DIM=64.
  v,k,q = split_heads(X) @ W{v,k,q}.T  (per-head, shared 64x64 weights)
  out   = softmax(q k^T / 8) v ; merge heads ; out @ Wo.T + bo

Sharding: core c -> batch b=c//4, query rows [qi*512, qi*512+512), qi=c%4.
Each core computes all 16 heads for its 512 query rows; no collectives,
disjoint row-slice gather on the host.

v3 key idea: fold the per-head projections on the HOST so the device loop
has no K/V projection work at all:
  - scores = xq (Wq^T Wk / 8) xk^T: M is folded into a single Q-side
    projection (m_bd block-diag), raw DMA'd xk^T is the S stationary
    operand directly. K partition-half dups come straight from extra DMAs
    of the same DRAM region (no DVE chain).
  - P (xv Wv^T) Wo_h^T = (P xv) (Wo_h Wv)^T: Wv folds into Wo, so PV
    consumes raw values; the host pre-builds the ones-augmented V layout
    [128 kp, 16 kb, 2 hp, 65] per (batch, pair) so ONE contiguous DMA per
    pair loads it (ones column -> PSUM row 64 = softmax denominator).
This drops per-chunk PE work below the exp period (ACT is the true floor:
128 exps x ~1.11us), removing the steady-state ACT stalls v2 had.

Tail (v2 lost ~20us here): all 8 final-fc tiles now get their
norm-independent matmuls (merged[6], merged[7] top half, identity*acc)
emitted in one batch that overlaps the last exps/PVs and keeps the PE busy
through the final norm chain (no HAM re-throttle); the last norm multiply
is emitted in 128-col chunks so the per-sb late matmuls + evac + out-DMA
start as soon as their slice of merged[7] lands.

PSUM: scores 2x[128,1024]f32 (4 banks) + po 2 + mix 2 = 8; the final fc
round reuses score banks as 4 psf halves + 2 mix + 2 po = 8 concurrent.
"""

import os
import sys

sys.path.insert(0, "/opt/trn_rl_repo")

import numpy as np

import concourse.bass as bass
import concourse.mybir as mybir
import concourse.tile as tile
from concourse import bacc
from concourse.bass_utils import run_bass_kernel_spmd

B = 2
S = 2048
E = 1024
H = 16
D = 64
SQ = 512          # query rows per core
NCORES = 8
NPAIR = 8         # head pairs
KBLK = 16         # 128-row key blocks
FP = mybir.dt.float32

KDT = os.environ.get("KERNEL_DT", "fp16")  # fp16 | bf16 | f32r | fp32


def build_nc(kdt=None):
    kdt = kdt or KDT
    MD = {"fp16": mybir.dt.float16, "bf16": mybir.dt.bfloat16,
          "f32r": mybir.dt.float32r, "fp32": FP}[kdt]  # matmul operand dtype
    nc = bacc.Bacc("TRN2", target_bir_lowering=False, debug=False)

    ident = nc.dram_tensor("ident", [128, 128], MD, kind="ExternalInput").ap()
    xq_t = nc.dram_tensor("xq_t", [E, SQ], MD, kind="ExternalInput").ap()
    xk_t = nc.dram_tensor("xk_t", [E, S], MD, kind="ExternalInput").ap()
    xv_aug = nc.dram_tensor("xv_aug", [128, NPAIR * KBLK * 130], MD,
                            kind="ExternalInput").ap()
    m_bd = nc.dram_tensor("m_bd", [128, 128], MD, kind="ExternalInput").ap()
    wo_t = nc.dram_tensor("wo_t", [E, E], MD, kind="ExternalInput").ap()
    bo = nc.dram_tensor("bo", [1, E], FP, kind="ExternalInput").ap()
    out = nc.dram_tensor("out", [SQ, E], FP, kind="ExternalOutput").ap()

    with tile.TileContext(nc) as tc:
        _body(tc, xq_t, xk_t, xv_aug, m_bd, wo_t, bo, ident, out, MD)
    nc.compile()
    return nc


def _body(tc, xq_t, xk_t, xv_aug, m_bd, wo_t, bo, ident, out, MD):
    """Software-pipelined emission; the Tile scheduler keeps per-engine
    FIFO order ~= emission order, so next-pair DMA loads, the Q
    projection, and the fc_out partial rounds are emitted INTERLEAVED
    into the attention chunk stream at fixed slots."""
    from contextlib import ExitStack
    nc = tc.nc
    Exp = mybir.ActivationFunctionType.Exp

    ctx = ExitStack()
    with ctx:
        wp = ctx.enter_context(tc.tile_pool(name="w", bufs=1))
        xkp = ctx.enter_context(tc.tile_pool(name="xk", bufs=3))
        kap = ctx.enter_context(tc.tile_pool(name="ka", bufs=3))  # dup A half
        kbp = ctx.enter_context(tc.tile_pool(name="kb", bufs=3))  # dup B half
        xvp = ctx.enter_context(tc.tile_pool(name="xv", bufs=3))
        xqp = ctx.enter_context(tc.tile_pool(name="xq", bufs=3))
        qtp = ctx.enter_context(tc.tile_pool(name="qt", bufs=2))
        qdp = ctx.enter_context(tc.tile_pool(name="qd", bufs=2))
        ptp = ctx.enter_context(tc.tile_pool(name="pt", bufs=6))
        mgp = ctx.enter_context(tc.tile_pool(name="mg", bufs=4))
        dnp = ctx.enter_context(tc.tile_pool(name="dn", bufs=4))
        acp = ctx.enter_context(tc.tile_pool(name="ac", bufs=1))
        obp = ctx.enter_context(tc.tile_pool(name="ob", bufs=8))
        ps_s = ctx.enter_context(tc.tile_pool(name="ps_s", bufs=2, space="PSUM"))
        ps_o = ctx.enter_context(tc.tile_pool(name="ps_o", bufs=2, space="PSUM"))
        ps_m = ctx.enter_context(tc.tile_pool(name="ps_m", bufs=2, space="PSUM"))

        # ---- weights / bias; startup critical path (m_bd -> qproj,
        # xk chunk0 -> first S -> first exp) front-loaded on the DMA rings --
        mw = wp.tile([128, 128], MD, tag="mw")
        nc.sync.dma_start(mw[:], m_bd)
        nbias = wp.tile([128, 1], FP, tag="nbias")
        nc.gpsimd.memset(nbias[:], -4.0)
        # dummy activation: preload the exp table set (~2.7us) during the
        # startup DMAs instead of on the first real exp
        warm = wp.tile([1, 8], FP, tag="warm")
        nc.gpsimd.memset(warm[:], 0.0)
        nc.scalar.activation(warm[:], warm[:], Exp, scale=1.0,
                             bias=nbias[0:1, 0:1])
        # dummy fp32 matmuls on junk data: ~3.4us of sustained PE activity
        # during the startup DMA wait flips the HAM clock gate to 8/8, so
        # the first REAL matmuls run at 2.4GHz instead of 1.2
        wmt = wp.tile([128, 512], FP, tag="wmt")
        nc.gpsimd.memset(wmt[:], 0.25)
        for w_ in range(2):
            psw = ps_m.tile([128, 512], FP, tag="mix", name="psw")
            nc.tensor.matmul(psw[:], lhsT=wmt[:, 0:128], rhs=wmt[:],
                             start=True, stop=True)

        wo_tiles = [wp.tile([128, E], MD, tag=f"wo{et}", name=f"wo{et}")
                    for et in range(8)]
        id_t = wp.tile([128, 128], MD, tag="id_t")
        # fc_out SBUF accumulators (fp16 so the final round can fold them
        # into PSUM via an identity-stationary matmul), one per output tile
        acc = [acp.tile([128, 512], MD, tag=f"acc{i}", name=f"acc{i}")
               for i in range(8)]
        merged = {}
        st = {}  # pipelined per-pair tiles

        def emit_loads(p, first=False):
            xq = xqp.tile([128, SQ], MD, tag="xq")
            nc.sync.dma_start(xq[:], xq_t[p * 128:(p + 1) * 128, :])
            xk = xkp.tile([128, S], MD, tag="xk")
            for ch in range(4):
                nc.sync.dma_start(xk[:, ch * 512:(ch + 1) * 512],
                                  xk_t[p * 128:(p + 1) * 128,
                                       ch * 512:(ch + 1) * 512])
            # partition-half dups of raw K^T straight from DRAM: the active
            # head's K^T must exist at BOTH partition halves for the
            # row-tiled S pairs. kdA = head A (rows 0:64) repeated at rows
            # 64:128 (used by hp0 kb-odd); kdB = head B at rows 0:64.
            if first:
                kdA = None  # pair 0 hp0 runs serial S in one row group
            else:
                kdA = kap.tile([128, S], MD, tag="kdA")
                for ch in range(2):
                    nc.sync.dma_start(
                        kdA[64:128, ch * 1024:(ch + 1) * 1024],
                        xk_t[p * 128:p * 128 + 64,
                             ch * 1024:(ch + 1) * 1024])
            kdB = kbp.tile([128, S], MD, tag="kdB")
            for ch in range(2):
                nc.sync.dma_start(
                    kdB[0:64, ch * 1024:(ch + 1) * 1024],
                    xk_t[p * 128 + 64:(p + 1) * 128,
                         ch * 1024:(ch + 1) * 1024])
            st["k", p] = (xk, kdA, kdB)
            # ones-augmented raw V, host-prepared layout [kb, hp, 65]
            v = xvp.tile([128, KBLK * 130], MD, tag="v")
            for ch in range(2):
                nc.sync.dma_start(
                    v[:, ch * 1040:(ch + 1) * 1040],
                    xv_aug[:, p * 2080 + ch * 1040:p * 2080 + (ch + 1) * 1040])
            st["v", p] = v
            st["x", p] = xq

        def emit_qproj(p, dups=True):
            xq = st["x", p]
            qt = qtp.tile([128, SQ], MD, tag="qt")
            psq = ps_m.tile([128, 512], FP, tag="mix")
            nc.tensor.matmul(psq[:], lhsT=mw[:], rhs=xq[:],
                             start=True, stop=True)
            nc.vector.tensor_copy(qt[:], psq[:])
            if dups:
                qdA = qdp.tile([128, SQ], MD, tag="qdA")
                nc.vector.tensor_copy(qdA[64:128, :], qt[0:64, :])
                qdB = qdp.tile([128, SQ], MD, tag="qdB")
                nc.vector.tensor_copy(qdB[0:64, :], qt[64:128, :])
            else:
                qdA = qdB = None
            st["q", p] = (qt, qdA, qdB)

        def emit_fc_tile(pa, pb, i):
            # one fc_out output tile: acc[i] (+)= merged[pa] @ wo[pa]
            #                                  + merged[pb] @ wo[pb]
            sb, nch = i // 2, i % 2
            psf_t = ps_m.tile([128, 512], FP, tag="mix", name="psf")
            psf = psf_t[:]
            nc.tensor.matmul(
                psf,
                lhsT=merged[pa][:, sb * 128:(sb + 1) * 128],
                rhs=wo_tiles[pa][:, nch * 512:(nch + 1) * 512],
                start=True, stop=False, skip_group_check=True)
            nc.tensor.matmul(
                psf,
                lhsT=merged[pb][:, sb * 128:(sb + 1) * 128],
                rhs=wo_tiles[pb][:, nch * 512:(nch + 1) * 512],
                start=False, stop=True, skip_group_check=True)
            if pa == 0:
                nc.vector.tensor_add(acc[i][:], psf,
                                     bo_b[:, nch * 512:(nch + 1) * 512])
            else:
                nc.vector.tensor_add(acc[i][:], acc[i][:], psf)

        # ---- final fc round: all 8 tiles concurrent (4 ps_s halves +
        # 2 ps_m + 2 ps_o), norm(7,hp1)-independent matmuls batched first so
        # the PE stays busy through the last norm chain; the late matmul of
        # tile i waits only on its 128-col chunk of the final norm multiply.
        psfs = {}
        bigs = {}

        def fc_final_early(i):
            sb, nch = i // 2, i % 2
            if i < 4:
                if i % 2 == 0:
                    big = ps_s.tile([128, 1024], FP, tag="s", name=f"fcf{i}")
                    bigs[i] = big
                    psf = big[:, 0:512]
                else:
                    psf = bigs[i - 1][:, 512:1024]
            elif i < 6:
                psf_t = ps_m.tile([128, 512], FP, tag="mix", name="psf")
                psf = psf_t[:]
            else:
                psf_t = ps_o.tile([128, 512], FP, tag="o", name=f"pso{i}")
                psf = psf_t[:]
            nc.tensor.matmul(
                psf, lhsT=merged[6][:, sb * 128:(sb + 1) * 128],
                rhs=wo_tiles[6][:, nch * 512:(nch + 1) * 512],
                start=True, stop=False, skip_group_check=True)
            nc.tensor.matmul(
                psf, lhsT=merged[7][0:64, sb * 128:(sb + 1) * 128],
                rhs=wo_tiles[7][0:64, nch * 512:(nch + 1) * 512],
                start=False, stop=False, skip_group_check=True)
            nc.tensor.matmul(
                psf, lhsT=id_t[:], rhs=acc[i][:],
                start=False, stop=False, skip_group_check=True)
            psfs[i] = psf

        def fc_final_late(i):
            sb, nch = i // 2, i % 2
            psf = psfs.pop(i)
            nc.tensor.matmul(
                psf, lhsT=merged[7][64:128, sb * 128:(sb + 1) * 128],
                rhs=wo_tiles[7][64:128, nch * 512:(nch + 1) * 512],
                start=False, stop=True, skip_group_check=True)
            ot = obp.tile([128, 512], FP, tag="ob")
            if i % 2 == 0:
                nc.scalar.copy(ot[:], psf)
            else:
                nc.vector.tensor_copy(ot[:], psf)
            nc.sync.dma_start(
                out[sb * 128:(sb + 1) * 128,
                    nch * 512:(nch + 1) * 512], ot[:])

        def emit_norm(p, hp, po, split=False):
            # normalize: denominator row 64 -> partition 0 via a standard
            # copy (honors AP partition offsets; custom-DVE recip and
            # gpsimd broadcast need input physically at partition 0),
            # then multiply po rows 0-63 straight from PSUM (base 0).
            # split=True (last norm only): ACT does the dn copy (it is idle
            # once the exps end) and the multiply is emitted in 128-col
            # chunks so each sb slice of merged[7] lands ASAP for the
            # final-fc late matmuls.
            mg = mgp.tile([128, SQ], MD, name=f"m{p}", tag="mg") \
                if hp == 0 else merged[p]
            merged[p] = mg
            dn = dnp.tile([1, 512], FP, tag="dn")
            if split:
                nc.scalar.copy(dn[0:1, :], po[64:65, :])
            else:
                nc.vector.tensor_copy(dn[0:1, :], po[64:65, :])
            dr = dnp.tile([1, 512], FP, tag="dr")
            nc.vector.reciprocal_approx_fast(dr[0:1, :], dn[0:1, :])
            db = dnp.tile([64, 512], FP, tag="db")
            nc.gpsimd.partition_broadcast(db[:], dr[0:1, :], channels=64)
            if split:
                for sb in range(4):
                    sl = slice(sb * 128, (sb + 1) * 128)
                    nc.vector.tensor_mul(mg[hp * 64:(hp + 1) * 64, sl],
                                         po[0:64, sl], db[:, sl])
            else:
                nc.vector.tensor_mul(mg[hp * 64:(hp + 1) * 64, :],
                                     po[0:64, :], db[:])

        # ---- prologue: pair 0 loads + qproj, pair 1 loads ----
        emit_loads(0, first=True)
        bo_row = wp.tile([1, E], FP, tag="bo_row")
        nc.sync.dma_start(bo_row[:], bo)
        bo_b = wp.tile([128, E], FP, tag="bo_b")
        nc.gpsimd.partition_broadcast(bo_b[:], bo_row[0:1, :], channels=128)
        emit_qproj(0, dups=False)
        # pair-0 B-side dup only: hp0 runs serial S in one row group (PE is
        # HAM-cold anyway) but hp1 (past the cold window) still row-tiles
        qt0_ = st["q", 0][0]
        qdB0 = qdp.tile([128, SQ], MD, tag="qdB")
        nc.vector.tensor_copy(qdB0[0:64, :], qt0_[64:128, :])
        st["q", 0] = (qt0_, None, qdB0)
        emit_loads(1)

        # ---- attention: one flat chunk stream, software-pipelined ONE
        # deep (S+exp at n, PV at n-1) - the next chunk's S pair enters the
        # PE FIFO ahead of the previous chunk's PV, keeping ACT fed across
        # chunk, head, and pair boundaries. Heads sequential; S matmuls
        # issued as row-tiled pairs (kb even in PE rows 0-63, kb odd in
        # 64-127, concurrent on HW); exp FD=1024; PV accumulates po.
        chunks = [(p, hp, c) for p in range(NPAIR) for hp in range(2)
                  for c in range(8)]
        NCH = len(chunks)
        pts, pos = {}, {}

        def stage_S_exp(n):
            p, hp, c = chunks[n]
            if p == 1 and hp == 0 and c == 0:
                # fc weights aren't needed until pair 2's fc round
                for et in range(8):
                    nc.sync.dma_start(wo_tiles[et][:],
                                      wo_t[et * 128:(et + 1) * 128, :])
                nc.sync.dma_start(id_t[:], ident)
            kt, kdA, kdB = st["k", p]
            qt, qdA, qdB = st["q", p]
            if kdA is None and hp == 0:
                # pair 0 head 0: no A-dups; both S matmuls in row group 0
                # (serial on PE - fine, startup is HAM-cold anyway)
                r0 = r1 = slice(0, 64)
                kt0 = kt1 = kt
                qt0 = qt1 = qt
            elif hp == 0:
                kt0, qt0, r0 = kt, qt, slice(0, 64)
                kt1, qt1, r1 = kdA, qdA, slice(64, 128)
            else:
                kt0, qt0, r0 = kdB, qdB, slice(0, 64)
                kt1, qt1, r1 = kt, qt, slice(64, 128)
            kb0, kb1 = 2 * c, 2 * c + 1
            ps = ps_s.tile([128, 1024], FP, tag="s", name=f"s{p}_{hp}_{c}")
            nc.tensor.matmul(
                ps[:, 0:512],
                lhsT=kt0[r0, kb0 * 128:(kb0 + 1) * 128],
                rhs=qt0[r0, :],
                start=True, stop=True)
            nc.tensor.matmul(
                ps[:, 512:1024],
                lhsT=kt1[r1, kb1 * 128:(kb1 + 1) * 128],
                rhs=qt1[r1, :],
                start=True, stop=True)
            # exp(s - 4): the /8 score scale is folded into m_bd on the
            # host; the -4 shift cancels in softmax and keeps max P ~= e^7
            # well inside fp16 range
            pt_ = ptp.tile([128, 1024], MD, name="pt_")
            nc.scalar.activation(pt_[:], ps[:], Exp,
                                 scale=1.0, bias=nbias[:])
            pts[n] = pt_

        def stage_pv(n):
            p, hp, c = chunks[n]
            if c == 0:
                pos[p, hp] = ps_o.tile([65, 512], FP, tag="o",
                                       name=f"po{p}_{hp}")
            po = pos[p, hp]
            v = st["v", p]
            pt_ = pts.pop(n)
            kb0, kb1 = 2 * c, 2 * c + 1
            nc.tensor.matmul(
                po[:],
                lhsT=v[:, kb0 * 130 + hp * 65:kb0 * 130 + hp * 65 + 65],
                rhs=pt_[:, 0:512],
                start=(c == 0), stop=False, skip_group_check=True)
            nc.tensor.matmul(
                po[:],
                lhsT=v[:, kb1 * 130 + hp * 65:kb1 * 130 + hp * 65 + 65],
                rhs=pt_[:, 512:1024],
                start=False, stop=(c == 7), skip_group_check=True)
            if c == 7:
                emit_norm(p, hp, pos.pop((p, hp)),
                          split=(p == NPAIR - 1 and hp == 1))

        def interleave(n):
            # next-pair loads/projection and fc-round tiles, one small
            # piece per chunk slot so no emission point exceeds the
            # per-chunk ACT budget (~1.1us)
            p, hp, c = chunks[n]
            t = hp * 8 + c
            if t == 1 and p < 6:
                emit_loads(p + 2)
                return
            # pair 0 defers its projection slot past the HAM-cold window
            tt = t - 4 if p == 0 else t
            if tt == 6 and p < 7:
                emit_qproj(p + 1)
            elif p == 7 and 2 <= t <= 5:
                # pair 7 has no next-pair work; run its share of the (4,5)
                # round in the early slots so the final exps stream without
                # interleaved fc work
                emit_fc_tile(4, 5, t + 2)
            elif 7 <= t <= 10 and 2 <= p < 7:
                # fc round for an earlier pair couple, 4 tiles on the even
                # pair and 4 on the odd pair, so no single pair carries the
                # whole 16-matmul round
                if p % 2 == 0:
                    emit_fc_tile(p - 2, p - 1, t - 7)
                else:
                    emit_fc_tile(p - 3, p - 2, t - 3)

        for n in range(NCH + 1):
            if n < NCH:
                stage_S_exp(n)
            if 0 <= n - 1 < NCH:
                stage_pv(n - 1)
            if n < NCH:
                interleave(n)

        # all 8 early groups in one batch (they overlap the last exps/PVs
        # and keep the PE hot through the norm chain), then the late
        # matmul + evac + out-DMA per tile as its norm chunk lands
        for i in range(8):
            fc_final_early(i)
        for i in range(8):
            fc_final_late(i)


# ---------------------------------------------------------------------------
# host side
# ---------------------------------------------------------------------------

_NC_CACHE = {}


def _get_nc():
    if KDT not in _NC_CACHE:
        _NC_CACHE[KDT] = build_nc(KDT)
    return _NC_CACHE[KDT]


def _np_dt():
    if KDT == "bf16":
        import ml_dtypes
        return ml_dtypes.bfloat16
    if KDT == "fp16":
        return np.float16
    return np.float32


def _bd(w):
    """128x128 block-diag of W.T (two copies)."""
    wt = np.ascontiguousarray(np.asarray(w).T.astype(np.float32))
    o = np.zeros((128, 128), np.float32)
    o[:64, :64] = wt
    o[64:, 64:] = wt
    return o


def kernel(values, keys, queries, Wv, Wk, Wq, Wo, bo):
    values = np.asarray(values, np.float32)
    keys = np.asarray(keys, np.float32)
    queries = np.asarray(queries, np.float32)
    Wv = np.asarray(Wv, np.float32)
    Wk = np.asarray(Wk, np.float32)
    Wq = np.asarray(Wq, np.float32)
    Wo = np.asarray(Wo, np.float32)

    dt = _np_dt()
    ident = np.eye(128, dtype=np.float32).astype(dt)
    # scores = xq (Wq^T Wk / sqrt(D)) xk^T  -> single Q-side projection
    M = (Wq.T @ Wk) / np.sqrt(np.float32(D))
    m_bd = _bd(M.T).astype(dt)          # blocks of M
    # (P xv) (Wo_h Wv)^T: fold Wv into Wo
    wo_f = np.zeros((E, E), np.float32)
    for h in range(H):
        wo_f[h * D:(h + 1) * D, :] = (Wo[:, h * D:(h + 1) * D] @ Wv).T
    wo_t = np.ascontiguousarray(wo_f).astype(dt)
    bo_r = np.ascontiguousarray(np.asarray(bo, np.float32).reshape(1, E))

    xk_t = [np.ascontiguousarray(keys[b].T).astype(dt) for b in range(B)]
    # ones-augmented raw V: [128 kp, NPAIR, KBLK, 2, 65] -> [128, NPAIR*2080]
    xv_aug = []
    for b in range(B):
        aug = np.ones((128, NPAIR, KBLK, 2, 65), np.float32)
        vals = values[b].reshape(KBLK, 128, NPAIR, 2, D)
        aug[:, :, :, :, :D] = vals.transpose(1, 2, 0, 3, 4)
        xv_aug.append(np.ascontiguousarray(
            aug.reshape(128, NPAIR * KBLK * 130)).astype(dt))

    in_maps = []
    for c in range(NCORES):
        b, qi = c // 4, c % 4
        in_maps.append({
            "xq_t": np.ascontiguousarray(
                queries[b, qi * SQ:(qi + 1) * SQ, :].T).astype(dt),
            "xk_t": xk_t[b],
            "xv_aug": xv_aug[b],
            "m_bd": m_bd,
            "wo_t": wo_t, "bo": bo_r, "ident": ident,
        })

    nc = _get_nc()
    res = run_bass_kernel_spmd(nc, in_maps, list(range(NCORES)),
                               trace=bool(int(os.environ.get("BASS_TRACE", "0"))))
    full = np.empty((B, S, E), np.float32)
    for c in range(NCORES):
        b, qi = c // 4, c % 4
        full[b, qi * SQ:(qi + 1) * SQ, :] = res.results[c]["out"]
    kernel.last_results = res
    return full


# revision 8
# speedup vs baseline: 1.1517x; 1.0339x over previous
"""Multi-head attention Trainium2 kernel (8-core SPMD), v4.

Problem: B=2, S=2048, EMBED=1024, HEADS=16, HEAD_DIM=64.
  v,k,q = split_heads(X) @ W{v,k,q}.T  (per-head, shared 64x64 weights)
  out   = softmax(q k^T / 8) v ; merge heads ; out @ Wo.T + bo

Sharding: core c -> batch b=c//4, query rows [qi*512, qi*512+512), qi=c%4.
Each core computes all 16 heads for its 512 query rows; no collectives,
disjoint row-slice gather on the host.

v3: host-folded weights: scores = xq (Wq^T Wk/8) xk^T (single Q-side
projection; raw xk is the S stationary operand; K partition-half dups via
extra DMAs), and P(xv Wv^T)Wo_h^T = (P xv)(Wo_h Wv)^T (no V projection;
host pre-builds the ones-augmented V layout, one DMA per pair). Per-chunk
PE work sits below the exp period, so the steady state is ACT-bound and
gap-free (measured: zero ACT stalls pairs 2-7).

v4 additions (trace-driven):
  - HAM reality: the PE ran at 1.2GHz until ts~33.6us (HAM un-throttle is
    NOT one 3.4us busy window; it needed ~20+us of uninterrupted PE
    activity, and any PE gap seems to restart the count). So: a stream of
    SMALL fp16 junk matmuls starts the moment its memset lands and
    bridges every startup PE gap (before qproj, during the qt cast,
    across the first two chunks' pipeline-fill bubbles).
  - ones column FIRST in the V layout: the softmax denominator lands in
    po row 0, so the per-half-pair norm is recip(po[0:1]) directly from
    PSUM (the cross-partition denominator copy is gone everywhere),
    then gpsimd broadcast, then multiply from po[1:65].
  - fc round (6) runs INSIDE pair 7's chunk slots t=6..13 (one matmul +
    DVE add per slot; merged[6] is ready by pair-7 chunk 3), so the
    post-stream block shrinks to m7-top + identity-fold matmuls.
  - final fc regrouped sb-major (4 tiles of [128 q, 1024 e]): psum = 2
    ps_s bufs + 2 ps_m + 2 ps_o; sb0/sb2 evacuate on ACT (idle after the
    exps; identity-matmul folds acc), sb1/sb3 on DVE via tensor_add with
    acc (no identity matmul). The last norm multiply is emitted in
    128-col chunks so late matmuls start as each sb slice lands.
  - fp16 output, one row-merged [128, 1024] DMA per sb: 512 descriptors
    total instead of 1024 (the v3 tail was DMA-descriptor-bound at
    ~85ns/descriptor), and half the bytes.
"""

import os
import sys

sys.path.insert(0, "/opt/trn_rl_repo")

import numpy as np

import concourse.bass as bass
import concourse.mybir as mybir
import concourse.tile as tile
from concourse import bacc
from concourse.bass_utils import run_bass_kernel_spmd

B = 2
S = 2048
E = 1024
H = 16
D = 64
SQ = 512          # query rows per core
NCORES = 8
NPAIR = 8         # head pairs
KBLK = 16         # 128-row key blocks
FP = mybir.dt.float32

KDT = os.environ.get("KERNEL_DT", "fp16")  # fp16 | bf16 | f32r | fp32


def build_nc(kdt=None):
    kdt = kdt or KDT
    MD = {"fp16": mybir.dt.float16, "bf16": mybir.dt.bfloat16,
          "f32r": mybir.dt.float32r, "fp32": FP}[kdt]  # matmul operand dtype
    nc = bacc.Bacc("TRN2", target_bir_lowering=False, debug=False)

    ident = nc.dram_tensor("ident", [128, 128], MD, kind="ExternalInput").ap()
    xq_t = nc.dram_tensor("xq_t", [E, SQ], MD, kind="ExternalInput").ap()
    xk_t = nc.dram_tensor("xk_t", [E, S], MD, kind="ExternalInput").ap()
    xv_aug = nc.dram_tensor("xv_aug", [128, NPAIR * KBLK * 256], MD,
                            kind="ExternalInput").ap()
    m_bd = nc.dram_tensor("m_bd", [128, 128], MD, kind="ExternalInput").ap()
    wo_t = nc.dram_tensor("wo_t", [E, E], MD, kind="ExternalInput").ap()
    bo = nc.dram_tensor("bo", [1, E], FP, kind="ExternalInput").ap()
    out = nc.dram_tensor("out", [SQ, E], MD, kind="ExternalOutput").ap()

    with tile.TileContext(nc) as tc:
        _body(tc, xq_t, xk_t, xv_aug, m_bd, wo_t, bo, ident, out, MD)
    nc.compile()
    return nc


def _body(tc, xq_t, xk_t, xv_aug, m_bd, wo_t, bo, ident, out, MD):
    """Software-pipelined emission; the Tile scheduler keeps per-engine
    FIFO order ~= emission order, so next-pair DMA loads, the Q
    projection, and the fc_out partial rounds are emitted INTERLEAVED
    into the attention chunk stream at fixed slots."""
    from contextlib import ExitStack
    nc = tc.nc
    Exp = mybir.ActivationFunctionType.Exp

    ctx = ExitStack()
    with ctx:
        wp = ctx.enter_context(tc.tile_pool(name="w", bufs=1))
        xkp = ctx.enter_context(tc.tile_pool(name="xk", bufs=3))
        kap = ctx.enter_context(tc.tile_pool(name="ka", bufs=3))  # dup A half
        kbp = ctx.enter_context(tc.tile_pool(name="kb", bufs=3))  # dup B half
        xvp = ctx.enter_context(tc.tile_pool(name="xv", bufs=3))
        xqp = ctx.enter_context(tc.tile_pool(name="xq", bufs=3))
        qtp = ctx.enter_context(tc.tile_pool(name="qt", bufs=2))
        qdp = ctx.enter_context(tc.tile_pool(name="qd", bufs=2))
        ptp = ctx.enter_context(tc.tile_pool(name="pt", bufs=6))
        mgp = ctx.enter_context(tc.tile_pool(name="mg", bufs=4))
        dnp = ctx.enter_context(tc.tile_pool(name="dn", bufs=4))
        acp = ctx.enter_context(tc.tile_pool(name="ac", bufs=1))
        obp = ctx.enter_context(tc.tile_pool(name="ob", bufs=4))
        ps_s = ctx.enter_context(tc.tile_pool(name="ps_s", bufs=2, space="PSUM"))
        ps_o = ctx.enter_context(tc.tile_pool(name="ps_o", bufs=2, space="PSUM"))
        ps_m = ctx.enter_context(tc.tile_pool(name="ps_m", bufs=2, space="PSUM"))

        # ---- HAM priming: the junk-matmul tile's memset is emitted FIRST
        # so PE activity starts ASAP; junk mms bridge every startup PE gap
        # (HAM needs a long uninterrupted busy stretch to un-throttle).
        jnk = wp.tile([128, 256], MD, tag="jnk")
        nc.gpsimd.memset(jnk[:], 0.25)
        psj = ps_m.tile([128, 512], FP, tag="mix", name="psj")

        def emit_junk(k):
            for _ in range(k):
                nc.tensor.matmul(psj[:, 0:256], lhsT=jnk[:, 0:128],
                                 rhs=jnk[:], start=True, stop=True,
                                 skip_group_check=True)

        emit_junk(2)
        mw = wp.tile([128, 128], MD, tag="mw")
        nc.sync.dma_start(mw[:], m_bd)
        nbias = wp.tile([128, 1], FP, tag="nbias")
        nc.gpsimd.memset(nbias[:], -4.0)
        # dummy activation: preload the exp table set (~2.7us) during the
        # startup DMAs instead of on the first real exp
        warm = wp.tile([1, 8], FP, tag="warm")
        nc.gpsimd.memset(warm[:], 0.0)
        nc.scalar.activation(warm[:], warm[:], Exp, scale=1.0,
                             bias=nbias[0:1, 0:1])
        emit_junk(14)

        wo_tiles = [wp.tile([128, E], MD, tag=f"wo{et}", name=f"wo{et}")
                    for et in range(8)]
        id_t = wp.tile([128, 128], MD, tag="id_t")
        # fc_out SBUF accumulators (fp16 so the final round can fold them
        # into PSUM via an identity-stationary matmul), one per output tile
        acc = [acp.tile([128, 512], MD, tag=f"acc{i}", name=f"acc{i}")
               for i in range(8)]
        merged = {}
        st = {}  # pipelined per-pair tiles

        def emit_loads(p, first=False):
            xq = xqp.tile([128, SQ], MD, tag="xq")
            nc.sync.dma_start(xq[:], xq_t[p * 128:(p + 1) * 128, :])
            xk = xkp.tile([128, S], MD, tag="xk")
            for ch in range(4):
                nc.sync.dma_start(xk[:, ch * 512:(ch + 1) * 512],
                                  xk_t[p * 128:(p + 1) * 128,
                                       ch * 512:(ch + 1) * 512])
            # partition-half dups of raw K^T straight from DRAM: the active
            # head's K^T must exist at BOTH partition halves for the
            # row-tiled S pairs. kdA = head A (rows 0:64) repeated at rows
            # 64:128 (used by hp0 kb-odd); kdB = head B at rows 0:64.
            if first:
                kdA = None  # pair 0 hp0 runs serial S in one row group
            else:
                kdA = kap.tile([128, S], MD, tag="kdA")
                for ch in range(2):
                    nc.sync.dma_start(
                        kdA[64:128, ch * 1024:(ch + 1) * 1024],
                        xk_t[p * 128:p * 128 + 64,
                             ch * 1024:(ch + 1) * 1024])
            kdB = kbp.tile([128, S], MD, tag="kdB")
            for ch in range(2):
                nc.sync.dma_start(
                    kdB[0:64, ch * 1024:(ch + 1) * 1024],
                    xk_t[p * 128 + 64:(p + 1) * 128,
                         ch * 1024:(ch + 1) * 1024])
            st["k", p] = (xk, kdA, kdB)
            # ones-augmented raw V, host layout [kb, hp, 128]: ones at
            # col 0 (softmax denominator -> po partition 0, recip-legal),
            # data at cols 64:128 (32-aligned partition offset for the
            # norm multiply; [128,128] stationary = standard FWL path)
            v = xvp.tile([128, KBLK * 256], MD, tag="v")
            for ch in range(2):
                nc.sync.dma_start(
                    v[:, ch * 2048:(ch + 1) * 2048],
                    xv_aug[:, p * 4096 + ch * 2048:p * 4096 + (ch + 1) * 2048])
            st["v", p] = v
            st["x", p] = xq

        def emit_qproj(p, dups=True):
            xq = st["x", p]
            qt = qtp.tile([128, SQ], MD, tag="qt")
            psq = ps_m.tile([128, 512], FP, tag="mix")
            nc.tensor.matmul(psq[:], lhsT=mw[:], rhs=xq[:],
                             start=True, stop=True)
            nc.vector.tensor_copy(qt[:], psq[:])
            if dups:
                qdA = qdp.tile([128, SQ], MD, tag="qdA")
                nc.vector.tensor_copy(qdA[64:128, :], qt[0:64, :])
                qdB = qdp.tile([128, SQ], MD, tag="qdB")
                nc.vector.tensor_copy(qdB[0:64, :], qt[64:128, :])
            else:
                qdA = qdB = None
            st["q", p] = (qt, qdA, qdB)

        def emit_fc_tile(pa, pb, i):
            # one fc_out partial tile: acc[i] (+)= merged[pa] @ wo[pa]
            #                                   + merged[pb] @ wo[pb]
            sb, nch = i // 2, i % 2
            psf_t = ps_m.tile([128, 512], FP, tag="mix", name="psf")
            psf = psf_t[:]
            nc.tensor.matmul(
                psf,
                lhsT=merged[pa][:, sb * 128:(sb + 1) * 128],
                rhs=wo_tiles[pa][:, nch * 512:(nch + 1) * 512],
                start=True, stop=False, skip_group_check=True)
            nc.tensor.matmul(
                psf,
                lhsT=merged[pb][:, sb * 128:(sb + 1) * 128],
                rhs=wo_tiles[pb][:, nch * 512:(nch + 1) * 512],
                start=False, stop=True, skip_group_check=True)
            if pa == 0:
                nc.vector.tensor_add(acc[i][:], psf,
                                     bo_b[:, nch * 512:(nch + 1) * 512])
            else:
                nc.vector.tensor_add(acc[i][:], acc[i][:], psf)

        def emit_fc_tile6(i):
            # merged[6]'s fc contribution, one tile per pair-7 chunk slot
            sb, nch = i // 2, i % 2
            psf_t = ps_m.tile([128, 512], FP, tag="mix", name="psf")
            psf = psf_t[:]
            nc.tensor.matmul(
                psf,
                lhsT=merged[6][:, sb * 128:(sb + 1) * 128],
                rhs=wo_tiles[6][:, nch * 512:(nch + 1) * 512],
                start=True, stop=True, skip_group_check=True)
            nc.vector.tensor_add(acc[i][:], acc[i][:], psf)

        # ---- final fc, sb-major: tile sb = out rows [sb*128, +128), all
        # 1024 cols. psum: sb0/sb1 on the two ps_s bufs ([128,1024]), sb2
        # on 2 ps_m halves, sb3 on 2 ps_o halves (frees last, used last).
        # early = m7-top (+ identity acc-fold for the ACT-evac'd sb0/sb2);
        # late = m7-bottom per sb as its 128-col norm chunk lands, then
        # evac (ACT copy for sb0/sb2; DVE tensor_add folding acc for
        # sb1/sb3) and ONE row-merged fp16 out-DMA per sb.
        psfs = {}

        def fc_final_early(sb):
            if sb < 2:
                big = ps_s.tile([128, 1024], FP, tag="s", name=f"fcf{sb}")
                halves = (big[:, 0:512], big[:, 512:1024])
            else:
                pool = ps_m if sb == 2 else ps_o
                tag = "mix" if sb == 2 else "o"
                big = None
                halves = tuple(
                    pool.tile([128, 512], FP, tag=tag, name=f"fcf{sb}_{n}")[:]
                    for n in range(2))
            for nch in range(2):
                psf = halves[nch]
                nc.tensor.matmul(
                    psf, lhsT=merged[7][0:64, sb * 128:(sb + 1) * 128],
                    rhs=wo_tiles[7][0:64, nch * 512:(nch + 1) * 512],
                    start=True, stop=False, skip_group_check=True)
                if sb % 2 == 0:  # ACT evac: fold acc via identity matmul
                    nc.tensor.matmul(
                        psf, lhsT=id_t[:], rhs=acc[sb * 2 + nch][:],
                        start=False, stop=False, skip_group_check=True)
            psfs[sb] = (big if sb < 2 else None, halves)

        def fc_final_late(sb):
            big, halves = psfs.pop(sb)
            for nch in range(2):
                nc.tensor.matmul(
                    halves[nch],
                    lhsT=merged[7][64:128, sb * 128:(sb + 1) * 128],
                    rhs=wo_tiles[7][64:128, nch * 512:(nch + 1) * 512],
                    start=False, stop=True, skip_group_check=True)
            ot = obp.tile([128, E], MD, tag="ob")
            if sb == 0:
                nc.scalar.copy(ot[:], big[:])
            elif sb == 2:
                nc.scalar.copy(ot[:, 0:512], halves[0])
                nc.scalar.copy(ot[:, 512:1024], halves[1])
            else:
                for nch in range(2):
                    nc.vector.tensor_add(ot[:, nch * 512:(nch + 1) * 512],
                                         halves[nch], acc[sb * 2 + nch][:])
            nc.sync.dma_start(out[sb * 128:(sb + 1) * 128, :], ot[:])

        def emit_norm(p, hp, po, split=False):
            # normalize: denominator is po row 0 (ones-first V layout), so
            # recip reads PSUM partition 0 directly; gpsimd broadcasts to
            # 64 partitions; multiply streams po rows 1:65 out of PSUM.
            # split=True (last norm only): multiply in 128-col chunks so
            # each sb slice of merged[7] lands ASAP for the final-fc lates.
            mg = mgp.tile([128, SQ], MD, name=f"m{p}", tag="mg") \
                if hp == 0 else merged[p]
            merged[p] = mg
            dr = dnp.tile([1, 512], FP, tag="dr")
            nc.vector.reciprocal_approx_fast(dr[0:1, :], po[0:1, :])
            db = dnp.tile([64, 512], FP, tag="db")
            nc.gpsimd.partition_broadcast(db[:], dr[0:1, :], channels=64)
            if split:
                for sb in range(4):
                    sl = slice(sb * 128, (sb + 1) * 128)
                    nc.vector.tensor_mul(mg[hp * 64:(hp + 1) * 64, sl],
                                         po[64:128, sl], db[:, sl])
            else:
                nc.vector.tensor_mul(mg[hp * 64:(hp + 1) * 64, :],
                                     po[64:128, :], db[:])

        # ---- prologue: pair 0 loads + qproj, pair 1 loads ----
        emit_loads(0, first=True)
        bo_row = wp.tile([1, E], FP, tag="bo_row")
        nc.sync.dma_start(bo_row[:], bo)
        bo_b = wp.tile([128, E], FP, tag="bo_b")
        nc.gpsimd.partition_broadcast(bo_b[:], bo_row[0:1, :], channels=128)
        emit_qproj(0, dups=False)
        emit_junk(5)  # bridge the qt-cast window before the first S pair
        # pair-0 B-side dup only: hp0 runs serial S in one row group (PE is
        # HAM-cold anyway) but hp1 (past the cold window) still row-tiles
        qt0_ = st["q", 0][0]
        qdB0 = qdp.tile([128, SQ], MD, tag="qdB")
        nc.vector.tensor_copy(qdB0[0:64, :], qt0_[64:128, :])
        st["q", 0] = (qt0_, None, qdB0)
        emit_loads(1)

        # ---- attention: one flat chunk stream, software-pipelined ONE
        # deep (S+exp at n, PV at n-1) - the next chunk's S pair enters the
        # PE FIFO ahead of the previous chunk's PV, keeping ACT fed across
        # chunk, head, and pair boundaries. Heads sequential; S matmuls
        # issued as row-tiled pairs (kb even in PE rows 0-63, kb odd in
        # 64-127, concurrent on HW); exp FD=1024; PV accumulates po.
        chunks = [(p, hp, c) for p in range(NPAIR) for hp in range(2)
                  for c in range(8)]
        NCH = len(chunks)
        pts, pos = {}, {}

        def stage_S_exp(n):
            p, hp, c = chunks[n]
            if p == 1 and hp == 0 and c == 0:
                # fc weights aren't needed until pair 2's fc round
                for et in range(8):
                    nc.sync.dma_start(wo_tiles[et][:],
                                      wo_t[et * 128:(et + 1) * 128, :])
                nc.sync.dma_start(id_t[:], ident)
            kt, kdA, kdB = st["k", p]
            qt, qdA, qdB = st["q", p]
            if kdA is None and hp == 0:
                # pair 0 head 0: no A-dups; both S matmuls in row group 0
                # (serial on PE - fine, startup is HAM-cold anyway)
                r0 = r1 = slice(0, 64)
                kt0 = kt1 = kt
                qt0 = qt1 = qt
            elif hp == 0:
                kt0, qt0, r0 = kt, qt, slice(0, 64)
                kt1, qt1, r1 = kdA, qdA, slice(64, 128)
            else:
                kt0, qt0, r0 = kdB, qdB, slice(0, 64)
                kt1, qt1, r1 = kt, qt, slice(64, 128)
            kb0, kb1 = 2 * c, 2 * c + 1
            ps = ps_s.tile([128, 1024], FP, tag="s", name=f"s{p}_{hp}_{c}")
            nc.tensor.matmul(
                ps[:, 0:512],
                lhsT=kt0[r0, kb0 * 128:(kb0 + 1) * 128],
                rhs=qt0[r0, :],
                start=True, stop=True)
            nc.tensor.matmul(
                ps[:, 512:1024],
                lhsT=kt1[r1, kb1 * 128:(kb1 + 1) * 128],
                rhs=qt1[r1, :],
                start=True, stop=True)
            # exp(s - 4): the /8 score scale is folded into m_bd on the
            # host; the -4 shift cancels in softmax and keeps max P ~= e^7
            # well inside fp16 range
            pt_ = ptp.tile([128, 1024], MD, name="pt_")
            nc.scalar.activation(pt_[:], ps[:], Exp,
                                 scale=1.0, bias=nbias[:])
            pts[n] = pt_
            if n < 2:
                # bridge the pipeline-fill PE bubble (PV n waits exp n);
                # junk sits BEFORE the blocked PV in the in-order PE FIFO
                emit_junk(4)

        def stage_pv(n):
            p, hp, c = chunks[n]
            if c == 0:
                pos[p, hp] = ps_o.tile([128, 512], FP, tag="o",
                                       name=f"po{p}_{hp}")
            po = pos[p, hp]
            v = st["v", p]
            pt_ = pts.pop(n)
            kb0, kb1 = 2 * c, 2 * c + 1
            nc.tensor.matmul(
                po[:],
                lhsT=v[:, kb0 * 256 + hp * 128:kb0 * 256 + hp * 128 + 128],
                rhs=pt_[:, 0:512],
                start=(c == 0), stop=False, skip_group_check=True)
            nc.tensor.matmul(
                po[:],
                lhsT=v[:, kb1 * 256 + hp * 128:kb1 * 256 + hp * 128 + 128],
                rhs=pt_[:, 512:1024],
                start=False, stop=(c == 7), skip_group_check=True)
            if c == 7:
                emit_norm(p, hp, pos.pop((p, hp)),
                          split=(p == NPAIR - 1 and hp == 1))

        def interleave(n):
            # next-pair loads/projection and fc-round tiles, one small
            # piece per chunk slot so no emission point exceeds the
            # per-chunk ACT budget (~1.1us)
            p, hp, c = chunks[n]
            t = hp * 8 + c
            if t == 1 and p < 6:
                emit_loads(p + 2)
                return
            # pair 0 defers its projection slot past the HAM-cold window
            tt = t - 4 if p == 0 else t
            if tt == 6 and p < 7:
                emit_qproj(p + 1)
            elif p == 7 and 2 <= t <= 5:
                # pair 7 has no next-pair work; run its share of the (4,5)
                # round in the early slots so the final exps stream without
                # a double fc burden
                emit_fc_tile(4, 5, t + 2)
            elif p == 7 and 6 <= t <= 13:
                # merged[6]'s fc contribution, one tile per slot
                emit_fc_tile6(t - 6)
            elif 7 <= t <= 10 and 2 <= p < 7:
                # fc round for an earlier pair couple, 4 tiles on the even
                # pair and 4 on the odd pair, so no single pair carries the
                # whole 16-matmul round
                if p % 2 == 0:
                    emit_fc_tile(p - 2, p - 1, t - 7)
                else:
                    emit_fc_tile(p - 3, p - 2, t - 3)

        for n in range(NCH + 1):
            if n < NCH:
                stage_S_exp(n)
            if 0 <= n - 1 < NCH:
                stage_pv(n - 1)
            if n < NCH:
                interleave(n)

        # sb0-2's early groups overlap the last exps/PVs and the norm
        # chain; each late runs as its norm chunk lands; sb3 (on the
        # ps_o slots) can only start once the norm has fully read po.
        for sb in range(3):
            fc_final_early(sb)
        for sb in range(3):
            fc_final_late(sb)
        fc_final_early(3)
        fc_final_late(3)


# ---------------------------------------------------------------------------
# host side
# ---------------------------------------------------------------------------

_NC_CACHE = {}


def _get_nc():
    if KDT not in _NC_CACHE:
        _NC_CACHE[KDT] = build_nc(KDT)
    return _NC_CACHE[KDT]


def _np_dt():
    if KDT == "bf16":
        import ml_dtypes
        return ml_dtypes.bfloat16
    if KDT == "fp16":
        return np.float16
    return np.float32


def _bd(w):
    """128x128 block-diag of W.T (two copies)."""
    wt = np.ascontiguousarray(np.asarray(w).T.astype(np.float32))
    o = np.zeros((128, 128), np.float32)
    o[:64, :64] = wt
    o[64:, 64:] = wt
    return o


def kernel(values, keys, queries, Wv, Wk, Wq, Wo, bo):
    values = np.asarray(values, np.float32)
    keys = np.asarray(keys, np.float32)
    queries = np.asarray(queries, np.float32)
    Wv = np.asarray(Wv, np.float32)
    Wk = np.asarray(Wk, np.float32)
    Wq = np.asarray(Wq, np.float32)
    Wo = np.asarray(Wo, np.float32)

    dt = _np_dt()
    ident = np.eye(128, dtype=np.float32).astype(dt)
    # scores = xq (Wq^T Wk / sqrt(D)) xk^T  -> single Q-side projection
    M = (Wq.T @ Wk) / np.sqrt(np.float32(D))
    m_bd = _bd(M.T).astype(dt)          # blocks of M
    # (P xv) (Wo_h Wv)^T: fold Wv into Wo
    wo_f = np.zeros((E, E), np.float32)
    for h in range(H):
        wo_f[h * D:(h + 1) * D, :] = (Wo[:, h * D:(h + 1) * D] @ Wv).T
    wo_t = np.ascontiguousarray(wo_f).astype(dt)
    bo_r = np.ascontiguousarray(np.asarray(bo, np.float32).reshape(1, E))

    xk_t = [np.ascontiguousarray(keys[b].T).astype(dt) for b in range(B)]
    # ones-augmented raw V: [128 kp, NPAIR, KBLK, 2, 128] with ones at
    # col 0 and data at cols 64:128 (see kernel-side comment)
    xv_aug = []
    for b in range(B):
        aug = np.zeros((128, NPAIR, KBLK, 2, 128), np.float32)
        aug[:, :, :, :, 0] = 1.0
        vals = values[b].reshape(KBLK, 128, NPAIR, 2, D)
        aug[:, :, :, :, 64:] = vals.transpose(1, 2, 0, 3, 4)
        xv_aug.append(np.ascontiguousarray(
            aug.reshape(128, NPAIR * KBLK * 256)).astype(dt))

    in_maps = []
    for c in range(NCORES):
        b, qi = c // 4, c % 4
        in_maps.append({
            "xq_t": np.ascontiguousarray(
                queries[b, qi * SQ:(qi + 1) * SQ, :].T).astype(dt),
            "xk_t": xk_t[b],
            "xv_aug": xv_aug[b],
            "m_bd": m_bd,
            "wo_t": wo_t, "bo": bo_r, "ident": ident,
        })

    nc = _get_nc()
    res = run_bass_kernel_spmd(nc, in_maps, list(range(NCORES)),
                               trace=bool(int(os.environ.get("BASS_TRACE", "0"))))
    full = np.empty((B, S, E), np.float32)
    for c in range(NCORES):
        b, qi = c // 4, c % 4
        full[b, qi * SQ:(qi + 1) * SQ, :] = \
            np.asarray(res.results[c]["out"], np.float32)
    kernel.last_results = res
    return full


# revision 9
# speedup vs baseline: 1.1766x; 1.0216x over previous
"""Multi-head attention Trainium2 kernel (8-core SPMD), v4.

Problem: B=2, S=2048, EMBED=1024, HEADS=16, HEAD_DIM=64.
  v,k,q = split_heads(X) @ W{v,k,q}.T  (per-head, shared 64x64 weights)
  out   = softmax(q k^T / 8) v ; merge heads ; out @ Wo.T + bo

Sharding: core c -> batch b=c//4, query rows [qi*512, qi*512+512), qi=c%4.
Each core computes all 16 heads for its 512 query rows; no collectives,
disjoint row-slice gather on the host.

v3: host-folded weights: scores = xq (Wq^T Wk/8) xk^T (single Q-side
projection; raw xk is the S stationary operand; K partition-half dups via
extra DMAs), and P(xv Wv^T)Wo_h^T = (P xv)(Wo_h Wv)^T (no V projection;
host pre-builds the ones-augmented V layout, one DMA per pair). Per-chunk
PE work sits below the exp period, so the steady state is ACT-bound and
gap-free (measured: zero ACT stalls pairs 2-7).

v4 additions (trace-driven):
  - HAM reality: the PE ran at 1.2GHz until ts~33.6us (HAM un-throttle is
    NOT one 3.4us busy window; it needed ~20+us of uninterrupted PE
    activity, and any PE gap seems to restart the count). So: a stream of
    SMALL fp16 junk matmuls starts the moment its memset lands and
    bridges every startup PE gap (before qproj, during the qt cast,
    across the first two chunks' pipeline-fill bubbles).
  - ones column FIRST in the V layout: the softmax denominator lands in
    po row 0, so the per-half-pair norm is recip(po[0:1]) directly from
    PSUM (the cross-partition denominator copy is gone everywhere),
    then gpsimd broadcast, then multiply from po[1:65].
  - fc round (6) runs INSIDE pair 7's chunk slots t=6..13 (one matmul +
    DVE add per slot; merged[6] is ready by pair-7 chunk 3), so the
    post-stream block shrinks to m7-top + identity-fold matmuls.
  - final fc regrouped sb-major (4 tiles of [128 q, 1024 e]): psum = 2
    ps_s bufs + 2 ps_m + 2 ps_o; sb0/sb2 evacuate on ACT (idle after the
    exps; identity-matmul folds acc), sb1/sb3 on DVE via tensor_add with
    acc (no identity matmul). The last norm multiply is emitted in
    128-col chunks so late matmuls start as each sb slice lands.
  - fp16 output, one row-merged [128, 1024] DMA per sb: 512 descriptors
    total instead of 1024 (the v3 tail was DMA-descriptor-bound at
    ~85ns/descriptor), and half the bytes.
"""

import os
import sys

sys.path.insert(0, "/opt/trn_rl_repo")

import numpy as np

import concourse.bass as bass
import concourse.mybir as mybir
import concourse.tile as tile
from concourse import bacc
from concourse.bass_utils import run_bass_kernel_spmd

B = 2
S = 2048
E = 1024
H = 16
D = 64
SQ = 512          # query rows per core
NCORES = 8
NPAIR = 8         # head pairs
KBLK = 16         # 128-row key blocks
FP = mybir.dt.float32

KDT = os.environ.get("KERNEL_DT", "fp16")  # fp16 | bf16 | f32r | fp32


def build_nc(kdt=None):
    kdt = kdt or KDT
    MD = {"fp16": mybir.dt.float16, "bf16": mybir.dt.bfloat16,
          "f32r": mybir.dt.float32r, "fp32": FP}[kdt]  # matmul operand dtype
    nc = bacc.Bacc("TRN2", target_bir_lowering=False, debug=False)

    ident = nc.dram_tensor("ident", [128, 128], MD, kind="ExternalInput").ap()
    xq_t = nc.dram_tensor("xq_t", [E, SQ], MD, kind="ExternalInput").ap()
    xk_t = nc.dram_tensor("xk_t", [E, S], MD, kind="ExternalInput").ap()
    xv_aug = nc.dram_tensor("xv_aug", [128, NPAIR * KBLK * 256], MD,
                            kind="ExternalInput").ap()
    m_bd = nc.dram_tensor("m_bd", [128, 128], MD, kind="ExternalInput").ap()
    wo_t = nc.dram_tensor("wo_t", [E, E], MD, kind="ExternalInput").ap()
    bo = nc.dram_tensor("bo", [1, E], FP, kind="ExternalInput").ap()
    out = nc.dram_tensor("out", [SQ, E], MD, kind="ExternalOutput").ap()

    with tile.TileContext(nc) as tc:
        _body(tc, xq_t, xk_t, xv_aug, m_bd, wo_t, bo, ident, out, MD)
    nc.compile()
    return nc


def _body(tc, xq_t, xk_t, xv_aug, m_bd, wo_t, bo, ident, out, MD):
    """Software-pipelined emission; the Tile scheduler keeps per-engine
    FIFO order ~= emission order, so next-pair DMA loads, the Q
    projection, and the fc_out partial rounds are emitted INTERLEAVED
    into the attention chunk stream at fixed slots."""
    from contextlib import ExitStack
    nc = tc.nc
    Exp = mybir.ActivationFunctionType.Exp

    ctx = ExitStack()
    with ctx:
        wp = ctx.enter_context(tc.tile_pool(name="w", bufs=1))
        xkp = ctx.enter_context(tc.tile_pool(name="xk", bufs=3))
        kap = ctx.enter_context(tc.tile_pool(name="ka", bufs=3))  # dup A half
        kbp = ctx.enter_context(tc.tile_pool(name="kb", bufs=3))  # dup B half
        xvp = ctx.enter_context(tc.tile_pool(name="xv", bufs=3))
        xqp = ctx.enter_context(tc.tile_pool(name="xq", bufs=3))
        qtp = ctx.enter_context(tc.tile_pool(name="qt", bufs=2))
        qdp = ctx.enter_context(tc.tile_pool(name="qd", bufs=2))
        ptp = ctx.enter_context(tc.tile_pool(name="pt", bufs=6))
        mgp = ctx.enter_context(tc.tile_pool(name="mg", bufs=4))
        dnp = ctx.enter_context(tc.tile_pool(name="dn", bufs=4))
        acp = ctx.enter_context(tc.tile_pool(name="ac", bufs=1))
        obp = ctx.enter_context(tc.tile_pool(name="ob", bufs=4))
        ps_s = ctx.enter_context(tc.tile_pool(name="ps_s", bufs=2, space="PSUM"))
        ps_o = ctx.enter_context(tc.tile_pool(name="ps_o", bufs=2, space="PSUM"))
        ps_m = ctx.enter_context(tc.tile_pool(name="ps_m", bufs=2, space="PSUM"))

        # ---- HAM priming: the junk-matmul tile's memset is emitted FIRST
        # so PE activity starts ASAP; junk mms bridge every startup PE gap
        # (HAM needs a long uninterrupted busy stretch to un-throttle).
        jnk = wp.tile([128, 256], MD, tag="jnk")
        nc.gpsimd.memset(jnk[:], 0.25)
        psj = ps_m.tile([128, 512], FP, tag="mix", name="psj")

        def emit_junk(k):
            for _ in range(k):
                nc.tensor.matmul(psj[:, 0:256], lhsT=jnk[:, 0:128],
                                 rhs=jnk[:], start=True, stop=True,
                                 skip_group_check=True)

        emit_junk(2)
        mw = wp.tile([128, 128], MD, tag="mw")
        nc.sync.dma_start(mw[:], m_bd)
        nbias = wp.tile([128, 1], FP, tag="nbias")
        nc.gpsimd.memset(nbias[:], -4.0)
        # dummy activation: preload the exp table set (~2.7us) during the
        # startup DMAs instead of on the first real exp
        warm = wp.tile([1, 8], FP, tag="warm")
        nc.gpsimd.memset(warm[:], 0.0)
        nc.scalar.activation(warm[:], warm[:], Exp, scale=1.0,
                             bias=nbias[0:1, 0:1])
        emit_junk(8)

        wo_tiles = [wp.tile([128, E], MD, tag=f"wo{et}", name=f"wo{et}")
                    for et in range(8)]
        id_t = wp.tile([128, 128], MD, tag="id_t")
        # fc_out SBUF accumulators (fp16 so the final round can fold them
        # into PSUM via an identity-stationary matmul), one per output tile
        acc = [acp.tile([128, 512], MD, tag=f"acc{i}", name=f"acc{i}")
               for i in range(8)]
        merged = {}
        st = {}  # pipelined per-pair tiles

        def emit_loads(p, first=False):
            # DMA order matches first-use order (matters for pair 0, which
            # races the startup: S chunk c needs xk cols [c*256,+256); PV
            # chunk c needs v cols [c*512,+512); the kd dups are first
            # used by hp0's kb-odd / hp1)
            xq = xqp.tile([128, SQ], MD, tag="xq")
            nc.sync.dma_start(xq[:], xq_t[p * 128:(p + 1) * 128, :])
            xk = xkp.tile([128, S], MD, tag="xk")
            # ones-augmented raw V, host layout [kb, hp, 128]: ones at
            # col 0 (softmax denominator -> po partition 0, recip-legal),
            # data at cols 64:128 (32-aligned partition offset for the
            # norm multiply; [128,128] stationary = standard FWL path)
            v = xvp.tile([128, KBLK * 256], MD, tag="v")
            for ch in range(4):
                nc.sync.dma_start(xk[:, ch * 512:(ch + 1) * 512],
                                  xk_t[p * 128:(p + 1) * 128,
                                       ch * 512:(ch + 1) * 512])
                nc.sync.dma_start(
                    v[:, ch * 1024:(ch + 1) * 1024],
                    xv_aug[:, p * 4096 + ch * 1024:p * 4096 + (ch + 1) * 1024])
            # partition-half dups of raw K^T straight from DRAM: the active
            # head's K^T must exist at BOTH partition halves for the
            # row-tiled S pairs. kdA = head A (rows 0:64) repeated at rows
            # 64:128 (used by hp0 kb-odd); kdB = head B at rows 0:64.
            if first:
                kdA = None  # pair 0 hp0 runs serial S in one row group
            else:
                kdA = kap.tile([128, S], MD, tag="kdA")
                for ch in range(2):
                    nc.sync.dma_start(
                        kdA[64:128, ch * 1024:(ch + 1) * 1024],
                        xk_t[p * 128:p * 128 + 64,
                             ch * 1024:(ch + 1) * 1024])
            kdB = kbp.tile([128, S], MD, tag="kdB")
            for ch in range(2):
                nc.sync.dma_start(
                    kdB[0:64, ch * 1024:(ch + 1) * 1024],
                    xk_t[p * 128 + 64:(p + 1) * 128,
                         ch * 1024:(ch + 1) * 1024])
            st["k", p] = (xk, kdA, kdB)
            st["v", p] = v
            st["x", p] = xq

        def emit_qproj(p, dups=True):
            xq = st["x", p]
            qt = qtp.tile([128, SQ], MD, tag="qt")
            psq = ps_m.tile([128, 512], FP, tag="mix")
            nc.tensor.matmul(psq[:], lhsT=mw[:], rhs=xq[:],
                             start=True, stop=True)
            nc.vector.tensor_copy(qt[:], psq[:])
            if dups:
                qdA = qdp.tile([128, SQ], MD, tag="qdA")
                nc.vector.tensor_copy(qdA[64:128, :], qt[0:64, :])
                qdB = qdp.tile([128, SQ], MD, tag="qdB")
                nc.vector.tensor_copy(qdB[0:64, :], qt[64:128, :])
            else:
                qdA = qdB = None
            st["q", p] = (qt, qdA, qdB)

        def emit_fc_tile(pa, pb, i):
            # one fc_out partial tile: acc[i] (+)= merged[pa] @ wo[pa]
            #                                   + merged[pb] @ wo[pb]
            sb, nch = i // 2, i % 2
            psf_t = ps_m.tile([128, 512], FP, tag="mix", name="psf")
            psf = psf_t[:]
            nc.tensor.matmul(
                psf,
                lhsT=merged[pa][:, sb * 128:(sb + 1) * 128],
                rhs=wo_tiles[pa][:, nch * 512:(nch + 1) * 512],
                start=True, stop=False, skip_group_check=True)
            nc.tensor.matmul(
                psf,
                lhsT=merged[pb][:, sb * 128:(sb + 1) * 128],
                rhs=wo_tiles[pb][:, nch * 512:(nch + 1) * 512],
                start=False, stop=True, skip_group_check=True)
            if pa == 0:
                nc.vector.tensor_add(acc[i][:], psf,
                                     bo_b[:, nch * 512:(nch + 1) * 512])
            else:
                nc.vector.tensor_add(acc[i][:], acc[i][:], psf)

        def emit_fc_tile6(i):
            # merged[6]'s fc contribution, one tile per pair-7 chunk slot
            sb, nch = i // 2, i % 2
            psf_t = ps_m.tile([128, 512], FP, tag="mix", name="psf")
            psf = psf_t[:]
            nc.tensor.matmul(
                psf,
                lhsT=merged[6][:, sb * 128:(sb + 1) * 128],
                rhs=wo_tiles[6][:, nch * 512:(nch + 1) * 512],
                start=True, stop=True, skip_group_check=True)
            nc.vector.tensor_add(acc[i][:], acc[i][:], psf)

        # ---- final fc, sb-major: tile sb = out rows [sb*128, +128), all
        # 1024 cols. psum: sb0/sb1 on the two ps_s bufs ([128,1024]), sb2
        # on 2 ps_m halves, sb3 on 2 ps_o halves (frees last, used last).
        # early = m7-top (+ identity acc-fold for the ACT-evac'd sb0/sb2);
        # late = m7-bottom per sb as its 128-col norm chunk lands, then
        # evac (ACT copy for sb0/sb2; DVE tensor_add folding acc for
        # sb1/sb3) and ONE row-merged fp16 out-DMA per sb.
        psfs = {}

        def fc_final_early(sb):
            if sb < 2:
                big = ps_s.tile([128, 1024], FP, tag="s", name=f"fcf{sb}")
                halves = (big[:, 0:512], big[:, 512:1024])
            else:
                pool = ps_m if sb == 2 else ps_o
                tag = "mix" if sb == 2 else "o"
                big = None
                halves = tuple(
                    pool.tile([128, 512], FP, tag=tag, name=f"fcf{sb}_{n}")[:]
                    for n in range(2))
            for nch in range(2):
                psf = halves[nch]
                nc.tensor.matmul(
                    psf, lhsT=merged[7][0:64, sb * 128:(sb + 1) * 128],
                    rhs=wo_tiles[7][0:64, nch * 512:(nch + 1) * 512],
                    start=True, stop=False, skip_group_check=True)
                if sb % 2 == 0:  # ACT evac: fold acc via identity matmul
                    nc.tensor.matmul(
                        psf, lhsT=id_t[:], rhs=acc[sb * 2 + nch][:],
                        start=False, stop=False, skip_group_check=True)
            psfs[sb] = (big if sb < 2 else None, halves)

        def fc_final_late(sb):
            big, halves = psfs.pop(sb)
            for nch in range(2):
                nc.tensor.matmul(
                    halves[nch],
                    lhsT=merged[7][64:128, sb * 128:(sb + 1) * 128],
                    rhs=wo_tiles[7][64:128, nch * 512:(nch + 1) * 512],
                    start=False, stop=True, skip_group_check=True)
            ot = obp.tile([128, E], MD, tag="ob")
            if sb == 0:
                nc.scalar.copy(ot[:], big[:])
            elif sb == 2:
                nc.scalar.copy(ot[:, 0:512], halves[0])
                nc.scalar.copy(ot[:, 512:1024], halves[1])
            else:
                for nch in range(2):
                    nc.vector.tensor_add(ot[:, nch * 512:(nch + 1) * 512],
                                         halves[nch], acc[sb * 2 + nch][:])
            nc.sync.dma_start(out[sb * 128:(sb + 1) * 128, :], ot[:])

        def emit_norm(p, hp, po, split=False):
            # normalize: denominator is po row 0 (ones-first V layout), so
            # recip reads PSUM partition 0 directly; gpsimd broadcasts to
            # 64 partitions; multiply streams po rows 1:65 out of PSUM.
            # split=True (last norm only): multiply in 128-col chunks so
            # each sb slice of merged[7] lands ASAP for the final-fc lates.
            mg = mgp.tile([128, SQ], MD, name=f"m{p}", tag="mg") \
                if hp == 0 else merged[p]
            merged[p] = mg
            dr = dnp.tile([1, 512], FP, tag="dr")
            nc.vector.reciprocal_approx_fast(dr[0:1, :], po[0:1, :])
            db = dnp.tile([64, 512], FP, tag="db")
            nc.gpsimd.partition_broadcast(db[:], dr[0:1, :], channels=64)
            if split:
                for sb in range(4):
                    sl = slice(sb * 128, (sb + 1) * 128)
                    nc.vector.tensor_mul(mg[hp * 64:(hp + 1) * 64, sl],
                                         po[64:128, sl], db[:, sl])
            else:
                nc.vector.tensor_mul(mg[hp * 64:(hp + 1) * 64, :],
                                     po[64:128, :], db[:])

        # ---- prologue: pair 0 loads + qproj, pair 1 loads ----
        bo_row = wp.tile([1, E], FP, tag="bo_row")
        nc.sync.dma_start(bo_row[:], bo)
        bo_b = wp.tile([128, E], FP, tag="bo_b")
        nc.gpsimd.partition_broadcast(bo_b[:], bo_row[0:1, :], channels=128)
        emit_loads(0, first=True)
        emit_qproj(0, dups=False)
        emit_junk(5)  # bridge the qt-cast window before the first S pair
        # pair-0 B-side dup only: hp0 runs serial S in one row group (PE is
        # HAM-cold anyway) but hp1 (past the cold window) still row-tiles
        qt0_ = st["q", 0][0]
        qdB0 = qdp.tile([128, SQ], MD, tag="qdB")
        nc.vector.tensor_copy(qdB0[0:64, :], qt0_[64:128, :])
        st["q", 0] = (qt0_, None, qdB0)
        emit_loads(1)

        # ---- attention: one flat chunk stream, software-pipelined ONE
        # deep (S+exp at n, PV at n-1) - the next chunk's S pair enters the
        # PE FIFO ahead of the previous chunk's PV, keeping ACT fed across
        # chunk, head, and pair boundaries. Heads sequential; S matmuls
        # issued as row-tiled pairs (kb even in PE rows 0-63, kb odd in
        # 64-127, concurrent on HW); exp FD=1024; PV accumulates po.
        chunks = [(p, hp, c) for p in range(NPAIR) for hp in range(2)
                  for c in range(8)]
        NCH = len(chunks)
        pts, pos = {}, {}

        def stage_S_exp(n):
            p, hp, c = chunks[n]
            if p == 1 and hp == 0 and c == 0:
                # fc weights aren't needed until pair 2's fc round
                for et in range(8):
                    nc.sync.dma_start(wo_tiles[et][:],
                                      wo_t[et * 128:(et + 1) * 128, :])
                nc.sync.dma_start(id_t[:], ident)
            kt, kdA, kdB = st["k", p]
            qt, qdA, qdB = st["q", p]
            if kdA is None and hp == 0:
                # pair 0 head 0: no A-dups; both S matmuls in row group 0
                # (serial on PE - fine, startup is HAM-cold anyway)
                r0 = r1 = slice(0, 64)
                kt0 = kt1 = kt
                qt0 = qt1 = qt
            elif hp == 0:
                kt0, qt0, r0 = kt, qt, slice(0, 64)
                kt1, qt1, r1 = kdA, qdA, slice(64, 128)
            else:
                kt0, qt0, r0 = kdB, qdB, slice(0, 64)
                kt1, qt1, r1 = kt, qt, slice(64, 128)
            kb0, kb1 = 2 * c, 2 * c + 1
            ps = ps_s.tile([128, 1024], FP, tag="s", name=f"s{p}_{hp}_{c}")
            nc.tensor.matmul(
                ps[:, 0:512],
                lhsT=kt0[r0, kb0 * 128:(kb0 + 1) * 128],
                rhs=qt0[r0, :],
                start=True, stop=True)
            nc.tensor.matmul(
                ps[:, 512:1024],
                lhsT=kt1[r1, kb1 * 128:(kb1 + 1) * 128],
                rhs=qt1[r1, :],
                start=True, stop=True)
            # exp(s - 4): the /8 score scale is folded into m_bd on the
            # host; the -4 shift cancels in softmax and keeps max P ~= e^7
            # well inside fp16 range
            pt_ = ptp.tile([128, 1024], MD, name="pt_")
            nc.scalar.activation(pt_[:], ps[:], Exp,
                                 scale=1.0, bias=nbias[:])
            pts[n] = pt_
            if n < 4:
                # bridge the pipeline-fill PE bubble (PV n waits exp n);
                # junk sits BEFORE the blocked PV in the in-order PE FIFO
                emit_junk(4 if n < 2 else 2)

        def stage_pv(n):
            p, hp, c = chunks[n]
            if c == 0:
                pos[p, hp] = ps_o.tile([128, 512], FP, tag="o",
                                       name=f"po{p}_{hp}")
            po = pos[p, hp]
            v = st["v", p]
            pt_ = pts.pop(n)
            kb0, kb1 = 2 * c, 2 * c + 1
            nc.tensor.matmul(
                po[:],
                lhsT=v[:, kb0 * 256 + hp * 128:kb0 * 256 + hp * 128 + 128],
                rhs=pt_[:, 0:512],
                start=(c == 0), stop=False, skip_group_check=True)
            nc.tensor.matmul(
                po[:],
                lhsT=v[:, kb1 * 256 + hp * 128:kb1 * 256 + hp * 128 + 128],
                rhs=pt_[:, 512:1024],
                start=False, stop=(c == 7), skip_group_check=True)
            if c == 7:
                emit_norm(p, hp, pos.pop((p, hp)),
                          split=(p == NPAIR - 1 and hp == 1))

        def interleave(n):
            # next-pair loads/projection and fc-round tiles, one small
            # piece per chunk slot so no emission point exceeds the
            # per-chunk ACT budget (~1.1us)
            p, hp, c = chunks[n]
            t = hp * 8 + c
            if t == 1 and p < 6:
                emit_loads(p + 2)
                return
            # pair 0 defers its projection slot past the HAM-cold window
            tt = t - 4 if p == 0 else t
            if tt == 6 and p < 7:
                emit_qproj(p + 1)
            elif p == 7 and 2 <= t <= 5:
                # pair 7 has no next-pair work; run its share of the (4,5)
                # round in the early slots so the final exps stream without
                # a double fc burden
                emit_fc_tile(4, 5, t + 2)
            elif p == 7 and 6 <= t <= 13:
                # merged[6]'s fc contribution, one tile per slot
                emit_fc_tile6(t - 6)
            elif 7 <= t <= 10 and 2 <= p < 7:
                # fc round for an earlier pair couple, 4 tiles on the even
                # pair and 4 on the odd pair, so no single pair carries the
                # whole 16-matmul round
                if p % 2 == 0:
                    emit_fc_tile(p - 2, p - 1, t - 7)
                else:
                    emit_fc_tile(p - 3, p - 2, t - 3)

        for n in range(NCH + 1):
            if n < NCH:
                stage_S_exp(n)
            if 0 <= n - 1 < NCH:
                stage_pv(n - 1)
            if n < NCH:
                interleave(n)

        # sb0-2's early groups overlap the last exps/PVs and the norm
        # chain; each late runs as its norm chunk lands; sb3 (on the
        # ps_o slots) can only start once the norm has fully read po.
        for sb in range(3):
            fc_final_early(sb)
        emit_junk(10)  # bridge recip+bcast so the late matmuls run warm
        for sb in range(3):
            fc_final_late(sb)
        fc_final_early(3)
        fc_final_late(3)


# ---------------------------------------------------------------------------
# host side
# ---------------------------------------------------------------------------

_NC_CACHE = {}


def _get_nc():
    if KDT not in _NC_CACHE:
        _NC_CACHE[KDT] = build_nc(KDT)
    return _NC_CACHE[KDT]


def _np_dt():
    if KDT == "bf16":
        import ml_dtypes
        return ml_dtypes.bfloat16
    if KDT == "fp16":
        return np.float16
    return np.float32


def _bd(w):
    """128x128 block-diag of W.T (two copies)."""
    wt = np.ascontiguousarray(np.asarray(w).T.astype(np.float32))
    o = np.zeros((128, 128), np.float32)
    o[:64, :64] = wt
    o[64:, 64:] = wt
    return o


def kernel(values, keys, queries, Wv, Wk, Wq, Wo, bo):
    values = np.asarray(values, np.float32)
    keys = np.asarray(keys, np.float32)
    queries = np.asarray(queries, np.float32)
    Wv = np.asarray(Wv, np.float32)
    Wk = np.asarray(Wk, np.float32)
    Wq = np.asarray(Wq, np.float32)
    Wo = np.asarray(Wo, np.float32)

    dt = _np_dt()
    ident = np.eye(128, dtype=np.float32).astype(dt)
    # scores = xq (Wq^T Wk / sqrt(D)) xk^T  -> single Q-side projection
    M = (Wq.T @ Wk) / np.sqrt(np.float32(D))
    m_bd = _bd(M.T).astype(dt)          # blocks of M
    # (P xv) (Wo_h Wv)^T: fold Wv into Wo
    wo_f = np.zeros((E, E), np.float32)
    for h in range(H):
        wo_f[h * D:(h + 1) * D, :] = (Wo[:, h * D:(h + 1) * D] @ Wv).T
    wo_t = np.ascontiguousarray(wo_f).astype(dt)
    bo_r = np.ascontiguousarray(np.asarray(bo, np.float32).reshape(1, E))

    xk_t = [np.ascontiguousarray(keys[b].T).astype(dt) for b in range(B)]
    # ones-augmented raw V: [128 kp, NPAIR, KBLK, 2, 128] with ones at
    # col 0 and data at cols 64:128 (see kernel-side comment)
    xv_aug = []
    for b in range(B):
        aug = np.zeros((128, NPAIR, KBLK, 2, 128), np.float32)
        aug[:, :, :, :, 0] = 1.0
        vals = values[b].reshape(KBLK, 128, NPAIR, 2, D)
        aug[:, :, :, :, 64:] = vals.transpose(1, 2, 0, 3, 4)
        xv_aug.append(np.ascontiguousarray(
            aug.reshape(128, NPAIR * KBLK * 256)).astype(dt))

    in_maps = []
    for c in range(NCORES):
        b, qi = c // 4, c % 4
        in_maps.append({
            "xq_t": np.ascontiguousarray(
                queries[b, qi * SQ:(qi + 1) * SQ, :].T).astype(dt),
            "xk_t": xk_t[b],
            "xv_aug": xv_aug[b],
            "m_bd": m_bd,
            "wo_t": wo_t, "bo": bo_r, "ident": ident,
        })

    nc = _get_nc()
    res = run_bass_kernel_spmd(nc, in_maps, list(range(NCORES)),
                               trace=bool(int(os.environ.get("BASS_TRACE", "0"))))
    full = np.empty((B, S, E), np.float32)
    for c in range(NCORES):
        b, qi = c // 4, c % 4
        full[b, qi * SQ:(qi + 1) * SQ, :] = \
            np.asarray(res.results[c]["out"], np.float32)
    kernel.last_results = res
    return full
